# revision 88
# baseline (speedup 1.0000x reference)
"""Trainium2 Bass kernel for nn_Extractor_Processor_75368086110414.

Windowed-attention transformer block (ViTDet-style) + ResBottleneckBlock,
data-parallel over batch across 8 NeuronCores (2 images per core).

Device layout: activations live TRANSPOSED in SBUF as [feature, token] with
features on partitions, tokens (row-major per image) on the free dim. All
matmuls put the contraction dim on partitions (W^T tiles pre-transposed on
host). LayerNorm statistics (over channels = partitions) are computed with
ones-matmuls on the tensor engine; per-token stats are broadcast back across
partitions via a DMA round-trip through DRAM scratch.

Attention avoids materializing any transposes: S^T = k q^T (+ decomposed
rel-pos bias injected through an indicator matmul accumulated in PSUM),
exp on ACT, out^T = (P^T)^T-free col-tiled PV matmuls, and softmax
denominators from ones-matmul column sums of P^T.
"""

import json
import sys
import types

import numpy as np
import ml_dtypes

import concourse.bass as bass
import concourse.tile as tile
from concourse import mybir
from concourse.vector_clock import ScopedClock
from concourse.masks import make_identity

F32 = mybir.dt.float32
BF16 = mybir.dt.bfloat16
AF = mybir.ActivationFunctionType
OP = mybir.AluOpType

# ---------------------------------------------------------------------------
# Patch 1: the pinned walrus rejects >1 sync wait per instruction. Split the
# kernel-tail drain's waits across a chain of drains, and post-process the
# BIR JSON to peel extra waits off any instruction onto injected NoOps.
# ---------------------------------------------------------------------------
MAX_WAITS = 1
_patched = False


def _drain_and_barrier(self, tick_clock, wait_clock):
    nc = self.nc
    drain_inst = nc.sync.drain()
    wait_clock.add_sem_waits(
        drain_inst.ins, ScopedClock({None: tick_clock.global_clock})
    )
    waits = list(drain_inst.ins.sync_info.on_wait)
    if len(waits) > MAX_WAITS:
        drain_inst.ins.sync_info = mybir.SyncInfo(
            on_wait=waits[:MAX_WAITS], on_update=[]
        )
        rest = waits[MAX_WAITS:]
        for i in range(0, len(rest), MAX_WAITS):
            extra = nc.sync.drain()
            extra.ins.sync_info = mybir.SyncInfo(
                on_wait=rest[i : i + MAX_WAITS], on_update=[]
            )
    nc.all_engine_barrier()
    assert self.sems is not None
    popped = nc._tile_sem_poison_stack.pop()
    assert popped is self._sem_poison
    nc.clear_and_free_semaphores(list(self.sems.allocated().values()))
    nc.all_engine_barrier()


def _split_waits_json(data: bytes) -> bytes:
    bj = json.loads(data)
    counter = [0]
    changed = False
    for fn in bj.get("functions", []):
        for bb in fn.get("blocks", []):
            insts = bb.get("instructions")
            if not insts:
                continue
            out = []
            for inst in insts:
                si = inst.get("sync_info")
                waits = si.get("on_wait") if si else None
                if waits and len(waits) > MAX_WAITS:
                    keep = waits[-MAX_WAITS:]
                    rest = waits[:-MAX_WAITS]
                    for i in range(0, len(rest), MAX_WAITS):
                        counter[0] += 1
                        out.append({
                            "debug": inst.get("debug"),
                            "engine": inst["engine"],
                            "ins": [],
                            "name": f"I-ws{counter[0]}",
                            "opcode": "NoOp",
                            "outs": [],
                            "sync_info": {
                                "on_wait": rest[i : i + MAX_WAITS],
                                "on_update": [],
                            },
                        })
                    si["on_wait"] = keep
                    changed = True
                out.append(inst)
            bb["instructions"] = out
    if not changed:
        return data
    return json.dumps(bj).encode()


def _apply_patches():
    global _patched
    if _patched:
        return
    _patched = True
    tile.TileContext._drain_and_barrier = _drain_and_barrier
    orig = bass.Bass.to_json_bytes
    bass.Bass.to_json_bytes = lambda self, *a, **kw: _split_waits_json(
        orig(self, *a, **kw)
    )
    # Patch 2: the agent image's antenv lacks axon_hooks; register a shim so
    # run_bass_kernel_spmd(trace=True) can find the NTFF profile hook.
    if "antenv.axon_hooks" not in sys.modules:
        try:
            from trn_agent_boot.trn_boot import _ntff_profile_via_ctypes

            hook = _ntff_profile_via_ctypes("/opt/axon/libaxon_pjrt.so")
        except Exception:
            hook = None
        mod = types.ModuleType("antenv.axon_hooks")
        mod.get_axon_ntff_profile_hook = lambda: hook
        mod.set_axon_ntff_profile_hook = lambda h: None
        sys.modules["antenv.axon_hooks"] = mod


_apply_patches()

# ---------------------------------------------------------------------------
# Problem geometry (hardcoded per spec)
# ---------------------------------------------------------------------------
C = 1024
NH = 16
HD = 64
WS = 14
MLP = 4096
B, HH, WW = 16, 28, 28
NCORES = 8
BS = B // NCORES          # images per core
T = BS * HH * WW          # 1568 tokens per core
TT = 392                  # token tile (one 14-row window-band of one image)
NTT = T // TT             # 4
NC_C = C // 128           # 8 c-tiles
N_WIN = BS * 4            # 8 windows per core
N_CHUNK = 16              # LN stat chunks of 98 tokens (window halves)


def _win_base(w):
    i, wh, ww = w // 4, (w // 2) % 2, w % 2
    return i * 784 + wh * 392 + ww * 14, (i, wh, ww)


def _r4(t):
    # [128, T] -> [p, i, wh, r, w28]
    return t.rearrange("p (i wh r w) -> p i wh r w", i=BS, wh=2, r=WS)


def win_view(t, w):
    i, wh, ww = w // 4, (w // 2) % 2, w % 2
    return _r4(t)[:, i, wh, :, ww * 14:(ww + 1) * 14]          # [p,14,14]


def half_view(t, w, u):
    i, wh, ww = w // 4, (w // 2) % 2, w % 2
    return _r4(t)[:, i, wh, u * 7:(u + 1) * 7, ww * 14:(ww + 1) * 14]  # [p,7,14]


def chunk_view(t, u):
    # flat contiguous 98-token chunk u (any token partition works for
    # per-token stats over channels)
    return t[:, u * 98:(u + 1) * 98]


def pair_view(t, i, wh, hf):
    # two chunks (ww=0,1) as [p, rr7, ww2, c14] but ordered (ww, rr, c) to
    # match a flat (ww-major) source stream
    r6 = t.rearrange("p (i wh hf rr ww c) -> p i wh hf rr ww c",
                     i=BS, wh=2, hf=2, rr=7, ww=2)
    return r6[:, i, wh, hf, :, :, :].transpose([0, 2, 1, 3])   # [p, ww2, rr7, c14]


# ---------------------------------------------------------------------------
# LayerNorm-over-partitions helper
# ---------------------------------------------------------------------------
def ln_chunk(nc, sbp, psp, src_tiles, sl, mb, rsb, inv_c, eps_tile):
    """Emit squares + ones-matmul stats + mean/rsqrt epilogue for one
    392-column chunk. Squares go to bf16 ring tiles so the sq-sum matmuls
    run at bf16 rate; the sum matmuls consume the source tiles directly."""
    ncti = len(src_tiles)
    w = sl.stop - sl.start
    F32R = mybir.dt.float32r
    ones_r = globals()["_ones_fr"].bitcast(F32R)
    ones_b = globals()["_ones_bf"]
    src_bf = src_tiles[0].dtype == BF16
    sqs = []
    for ci in range(ncti):
        sq = sbp.tile([128, w], BF16, tag=f"lnsq{ci}", name=f"lnsq{ci}",
                      bufs=2)
        nc.scalar.activation(out=sq, in_=src_tiles[ci][:, sl],
                             func=AF.Square)
        sqs.append(sq)
    ps_s = psp.tile([128, w], F32, tag="lnsum", name="lnsum", bufs=2)
    ps_q = psp.tile([128, w], F32, tag="lnsqp", name="lnsqp", bufs=2)
    for ci in range(ncti):
        nc.tensor.matmul(ps_s, ones_b if src_bf else ones_r,
                         src_tiles[ci][:, sl],
                         start=(ci == 0), stop=(ci == ncti - 1))
    for ci in range(ncti):
        nc.tensor.matmul(ps_q, ones_b, sqs[ci],
                         start=(ci == 0), stop=(ci == ncti - 1))
    nc.scalar.activation(out=mb[:, sl], in_=ps_s, func=AF.Copy,
                         scale=inv_c)
    msq = sbp.tile([128, w], F32, tag="lnmsq", name="lnmsq", bufs=2)
    nc.vector.tensor_mul(out=msq, in0=mb[:, sl], in1=mb[:, sl])
    sqm = sbp.tile([128, w], F32, tag="lnsqm", name="lnsqm", bufs=2)
    nc.scalar.activation(out=sqm, in_=ps_q, func=AF.Copy, scale=inv_c)
    ve = sbp.tile([128, w], F32, tag="lnve", name="lnve", bufs=2)
    nc.vector.tensor_sub(out=ve, in0=sqm, in1=msq)
    nc.scalar.activation(out=ve, in_=ve, func=AF.Ln, bias=eps_tile)
    nc.scalar.activation(out=rsb[:, sl], in_=ve, func=AF.Exp,
                         scale=-0.5)


def ln_stats(nc, pools, src_tiles, scratch, mb, rsb, inv_c, eps_tile, ident,
             apply_cb=None):
    sbp, psp = pools
    for t0 in range(0, T, 392):
        sl = slice(t0, t0 + 392)
        ln_chunk(nc, sbp, psp, src_tiles, sl, mb, rsb, inv_c, eps_tile)
        if apply_cb is not None:
            apply_cb(sl)


# ---------------------------------------------------------------------------
# Program builder
# ---------------------------------------------------------------------------
ones_f32 = None  # set inside build


def build_program(debug=False, zero_bias=False):
    global ones_f32
    nc = bass.Bass()

    def din(name, shape, dt=F32):
        return nc.declare_dram_parameter(name, shape, dt, isOutput=False)

    xT = din("xT", [C, T], BF16)
    qkvwT = din("qkvwT", [C, 3 * C], BF16)
    qkvb = din("qkvb", [3 * C])
    qkvbv = din("qkvbv", [1, C], BF16)
    projwT = din("projwT", [C, C], BF16)
    projb = din("projb", [C])
    fc1wT = din("fc1wT", [C, MLP], BF16)
    fc1b = din("fc1b", [MLP])
    fc2wT = din("fc2wT", [MLP, C], BF16)
    fc2b = din("fc2b", [C])
    c1wT = din("c1wT", [C, C // 2], BF16)
    c2wT = din("c2wT", [9, C // 2, C // 2], BF16)
    c3wT = din("c3wT", [C // 2, C], BF16)
    n1w = din("n1w", [C // 2])
    n1b = din("n1b", [C // 2])
    n2w = din("n2w", [C // 2])
    n2b = din("n2b", [C // 2])
    n3w = din("n3w", [C])
    n3b = din("n3b", [C])
    rhe = din("rhe", [128, 196], BF16)
    rwe = din("rwe", [128, 196], BF16)
    kindAi = din("kindAi", [46, T], BF16)
    kindBi = din("kindBi", [64, T], BF16)
    zpad = din("zpad", [18, T], BF16)
    yT = nc.declare_dram_parameter("yT", [C, T], F32, isOutput=True)
    dbg = {}
    if debug:
        for name, shape, dt in [
            ("d_xln1", [C, T], BF16), ("d_qk", [2 * C, T], BF16),
            ("d_v", [98, N_WIN * 2 * C], BF16), ("d_rel", [128, T], BF16),
            ("d_xattn", [C, T], BF16), ("d_x2", [C, T], F32),
            ("d_xln2", [C, T], BF16), ("d_h", [MLP, 784], BF16),
            ("d_x3", [C, T], F32),
            ("d_c1", [C // 2, T], F32), ("d_r1p", [C // 2, BS * 900 + 2], BF16),
            ("d_c2", [C // 2, T], F32), ("d_c3", [C, T], F32),
            ("d_mb", [128, T], F32), ("d_rsb", [128, T], F32),
        ]:
            dbg[name] = nc.declare_dram_parameter(name, shape, dt, isOutput=True)

    def dump(name, tiles, rows=128):
        if not debug:
            return
        d = dbg[name]
        for i, t in enumerate(tiles):
            nc.sync.dma_start(out=d[i * rows:(i + 1) * rows, :][0:rows, :],
                              in_=t[0:rows, :] if rows != 98 else t)


    scratch = nc.dram_tensor("scratch", [2 * 98 * N_CHUNK], F32)
    scratch2 = nc.dram_tensor("scratch2", [64 * 392], F32)

    _cms = {}

    def pool(name, bufs=1, side=None):
        cm = tc.tile_pool(name=name, bufs=bufs, side=side)
        p = cm.__enter__()
        _cms[id(p)] = cm
        return p

    def pspool(name, bufs=1):
        cm = tc.tile_pool(name=name, bufs=bufs, space="PSUM")
        p = cm.__enter__()
        _cms[id(p)] = cm
        return p

    def close(*pools_):
        for p in pools_:
            _cms.pop(id(p)).__exit__(None, None, None)

    with tile.TileContext(nc) as tc:
        g = pool("glob")
        global ones_f32
        ones_f32 = g.tile([128, 1], F32, tag="ones_f32", name="ones_f32")
        nc.vector.memset(ones_f32, 1.0)
        ones_bf = g.tile([128, 128], BF16, tag="ones_bf", name="ones_bf")
        nc.vector.memset(ones_bf, 1.0)
        ones_fr = g.tile([128, 128], F32, tag="ones_fr", name="ones_fr")
        nc.vector.memset(ones_fr, 1.0)
        globals()["_ones_fr"] = ones_fr
        globals()["_ones_bf"] = ones_bf
        eps5 = g.tile([128, 1], F32, tag="eps5", name="eps5")
        nc.vector.memset(eps5, 1e-5)
        eps6 = g.tile([128, 1], F32, tag="eps6", name="eps6")
        nc.vector.memset(eps6, 1e-6)
        ident = g.tile([128, 128], F32, tag="ident", name="ident")
        make_identity(nc, ident)

        pa = pool("pa")                       # xln1, lives through qkv
        pc = pool("pc")                       # xattn, lives thru proj
        pwq = pool("pwq")                     # qkv weights (DMA'd early)
        p1x = pool("p1x")                     # raw x, dies after apply
        p1 = pool("p1", bufs=2)
        pqkps = pspool("pqkps", bufs=1)       # qk/v chain ring (2 banks),
        p1ps = pspool("p1ps", bufs=2)         # coexists with LN1 psum
        F32R = mybir.dt.float32r
        xt = []
        for ci in range(NC_C):
            xt.append(p1x.tile([128, T], BF16, tag=f"xt{ci}", name=f"xt{ci}"))
        for half in range(2):
            sl = slice(half * 2 * TT, (half + 1) * 2 * TT)
            for ci in range(NC_C):
                nc.sync.dma_start(out=xt[ci][:, sl],
                                  in_=xT[ci * 128:(ci + 1) * 128, sl])
        # rel-pos tables first (tiny)
        rhe_sb = g.tile([128, 196], BF16, tag="rhe_sb", name="rhe_sb")
        nc.sync.dma_start(out=rhe_sb, in_=rhe[:, :])
        rwe_sb = g.tile([128, 196], BF16, tag="rwe_sb", name="rwe_sb")
        nc.sync.dma_start(out=rwe_sb, in_=rwe[:, :])
        # qkv weights next on the queue: quarters q0 (q heads 0-7) and q2
        # (k heads 0-7) first so qk(0) can start as soon as LN1 drains.
        wq = []
        for ci in range(NC_C):
            wq.append(pwq.tile([128, 2 * C], BF16, tag=f"wqk{ci}",
                               name=f"wqk{ci}"))
        wv = []
        for ci in range(NC_C):
            wv.append(pwq.tile([128, C], BF16, tag=f"wv{ci}", name=f"wv{ci}"))
        for quad in (0, 2):
            for ci in range(NC_C):
                nc.sync.dma_start(
                    out=wq[ci][:, quad * 512:(quad + 1) * 512],
                    in_=qkvwT[ci * 128:(ci + 1) * 128,
                              quad * 512:(quad + 1) * 512])

        def stage_bias(src, n, name):
            t = g.tile([128, n], F32, tag=name, name=name)
            nc.sync.dma_start(out=t, in_=src.rearrange("(o p) -> p o", p=128))
            return t

        qkvb_sb = stage_bias(qkvb, 24, "qkvb_sb")
        for ci in range(NC_C):
            nc.sync.dma_start(
                out=wv[ci], in_=qkvwT[ci * 128:(ci + 1) * 128, 2 * C:3 * C])
        for quad in (1, 3):
            for ci in range(NC_C):
                nc.sync.dma_start(
                    out=wq[ci][:, quad * 512:(quad + 1) * 512],
                    in_=qkvwT[ci * 128:(ci + 1) * 128,
                              quad * 512:(quad + 1) * 512])
        projb_sb = stage_bias(projb, 8, "projb_sb")

        # qrel/kind tiles; pad rows and indicator rows come straight from
        # host-prepared DRAM templates (zero engine time). qrel pad rows
        # must be zeroed: garbage bf16 can be Inf/NaN and 0 (stationary)
        # * Inf = NaN in the accumulation.
        qrelA = [pwq.tile([128, T], BF16, tag=f"qrelA{i}", name=f"qrelA{i}")
                 for i in range(2)]
        qrelB = [pwq.tile([128, T], BF16, tag=f"qrelB{i}", name=f"qrelB{i}")
                 for i in range(2)]
        kindA = [pwq.tile([128, T], BF16, tag=f"kindA{i}", name=f"kindA{i}")
                 for i in range(2)]
        kindB = [pwq.tile([128, T], BF16, tag=f"kindB{i}", name=f"kindB{i}")
                 for i in range(2)]
        for i in range(2):
            nc.sync.dma_start(out=kindA[i][64:110, :], in_=kindAi[:, :])
            nc.sync.dma_start(out=kindB[i][0:64, :], in_=kindBi[:, :])
            nc.sync.dma_start(out=qrelA[i][78:96, :], in_=zpad[:, :])
            nc.sync.dma_start(out=qrelB[i][14:32, :], in_=zpad[:, :])
            nc.sync.dma_start(out=qrelB[i][46:64, :], in_=zpad[:, :])
        fc1b_sb = stage_bias(fc1b, 32, "fc1b_sb")
        fc2b_sb = stage_bias(fc2b, 8, "fc2b_sb")
        n1w_sb = stage_bias(n1w, 4, "n1w_sb")
        n1b_sb = stage_bias(n1b, 4, "n1b_sb")
        n2w_sb = stage_bias(n2w, 4, "n2w_sb")
        n2b_sb = stage_bias(n2b, 4, "n2b_sb")
        n3w_sb = stage_bias(n3w, 8, "n3w_sb")
        n3b_sb = stage_bias(n3b, 8, "n3b_sb")

        mb = g.tile([128, T], F32, tag="mb", name="mb")
        rsb = g.tile([128, T], F32, tag="rsb", name="rsb")

        # ---------------- Phase 1: LN1 -> XLn1 (bf16), pipelined ----------
        # x arrives in per-chunk DMAs; stats + apply are emitted per 392-col
        # chunk so the first qk matmuls can start ~15us in.
        # xln1 is stored WINDOW-ORDERED: col = w*196 + r*14 + c
        xln1 = []
        for ci in range(NC_C):
            xln1.append(pa.tile([128, T], BF16, tag=f"xln1_{ci}",
                                name=f"xln1_{ci}"))

        def emit_qk(ot, tts=None, epi_dve=True):
            hp2 = ot % 8
            tA, tB = ((qrelA, qrelB) if ot < 8 else (kindA, kindB))
            for tt in (range(NTT) if tts is None else tts):
                s = slice(tt * TT, (tt + 1) * TT)
                pst = pqkps.tile([128, TT], F32, tag="bigps", name="qkps",
                                 bufs=2)
                for ci in range(NC_C):
                    nc.tensor.matmul(
                        pst, wq[ci][:, ot * 128:(ot + 1) * 128],
                        xln1[ci][:, s],
                        start=(ci == 0), stop=(ci == NC_C - 1))
                nc.scalar.activation(out=tA[hp2 % 2][0:64, s],
                                     in_=pst[0:64, :],
                                     func=AF.Identity,
                                     bias=qkvb_sb[0:64, ot:ot + 1])
                if zero_bias and epi_dve:
                    nc.vector.tensor_copy(out=tB[hp2 % 2][64:128, s],
                                          in_=pst[64:128, :])
                else:
                    nc.scalar.activation(out=tB[hp2 % 2][64:128, s],
                                         in_=pst[64:128, :],
                                         func=AF.Identity,
                                         bias=qkvb_sb[64:128, ot:ot + 1])

        for tt in range(NTT):
            sl = slice(tt * TT, (tt + 1) * TT)
            sqs = []
            for ci in range(NC_C):
                sq = p1.tile([128, TT], BF16, tag=f"lnsq{ci}",
                             name=f"lnsq{ci}", bufs=2)
                nc.scalar.activation(out=sq, in_=xt[ci][:, sl],
                                     func=AF.Square)
                sqs.append(sq)
            ps_s = p1ps.tile([128, TT], F32, tag="lnsum", name="lnsum", bufs=2)
            ps_q = p1ps.tile([128, TT], F32, tag="lnsqp", name="lnsqp", bufs=2)
            for ci in range(NC_C):
                nc.tensor.matmul(ps_s, ones_bf, xt[ci][:, sl],
                                 start=(ci == 0), stop=(ci == NC_C - 1))
            for ci in range(NC_C):
                nc.tensor.matmul(ps_q, ones_bf, sqs[ci],
                                 start=(ci == 0), stop=(ci == NC_C - 1))
            nc.scalar.activation(out=mb[:, sl], in_=ps_s, func=AF.Copy,
                                 scale=1.0 / C)
            msq = p1.tile([128, TT], F32, tag="lnmsq", name="lnmsq", bufs=2)
            nc.vector.tensor_mul(out=msq, in0=mb[:, sl], in1=mb[:, sl])
            sqm = p1.tile([128, TT], F32, tag="lnsqm", name="lnsqm", bufs=2)
            nc.scalar.activation(out=sqm, in_=ps_q, func=AF.Copy,
                                 scale=1.0 / C)
            ve = p1.tile([128, TT], F32, tag="lnve", name="lnve", bufs=2)
            nc.vector.tensor_sub(out=ve, in0=sqm, in1=msq)
            nc.scalar.activation(out=ve, in_=ve, func=AF.Ln, bias=eps5)
            nc.scalar.activation(out=rsb[:, sl], in_=ve, func=AF.Exp,
                                 scale=-0.5)
            # apply: row-major (r, ww, c) -> window-ordered (ww, r, c)
            for ci in range(NC_C):
                tmp = p1.tile([128, TT], F32, tag="lntmp", name="lntmp",
                              bufs=3)
                nc.vector.tensor_sub(out=tmp, in0=xt[ci][:, sl],
                                     in1=mb[:, sl])
                nc.vector.tensor_mul(
                    out=xln1[ci][:, sl].rearrange(
                        "p (w r c) -> p w r c", w=2, r=WS),
                    in0=tmp.rearrange("p (r w c) -> p w r c", w=2, r=WS),
                    in1=rsb[:, sl].rearrange("p (r w c) -> p w r c",
                                             w=2, r=WS))
            # fuse the first head-pair's qk chains chunk-by-chunk so their
            # epilogues interleave with the LN1 applies on DVE/ACT
            emit_qk(0, tts=[tt])
            emit_qk(8, tts=[tt])
        dump("d_xln1", xln1)
        dump("d_mb", [mb])
        dump("d_rsb", [rsb])
        close(p1ps, p1, p1x)

        # ------- Block 1: qkv + attention, software-pipelined -------
        # Phase hp emits: qk matmuls for hp+1, then the window loop for hp
        # with rel-pos writeback for hp+1 (and v chains for s=1 on phases
        # 1-3) interleaved so the PE stream stays dense.
        pqk = pool("pqk", side="right")       # rotating qk tiles
        pvv = pool("pvv", side="right")       # v slices
        p2 = pool("p2", bufs=2)               # rel/pts/den tmps
        p2ps = pspool("p2ps", bufs=1)
        bvrow = pwq.tile([1, C], BF16, tag="bvrow", name="bvrow")
        nc.sync.dma_start(out=bvrow, in_=qkvbv[:, :])
        ones_row = pwq.tile([1, 98], BF16, tag="ones_row", name="ones_row")
        nc.vector.memset(ones_row, 1.0)

        xattn = []
        for hp in range(8):
            xattn.append(pc.tile([128, T], BF16, tag=f"xattn{hp}",
                                 name=f"xattn{hp}"))

        # kqind merge: stationary kind = [k rows | ind rows], moving qrel =
        # [q rows | rel rows]; one K=110/128 matmul replaces kq+ind pairs.
        #   qrelA/kindA rows: 0:64 q/k, 64:78 kh, 78:96 zero, 96:110 kw
        #   qrelB/kindB rows: 0:14 kh, 32:46 kw, 46:64 zero, 64:128 q/k
        def v_alloc(s):
            return pvv.tile([98, 16 * 512], BF16, tag=f"v{s}", name=f"v{s}")

        def emit_v_chain(s, vt, w, u):
            pv = pqkps.tile([98, 512], F32, tag="bigps", name="vps",
                            bufs=2)
            for ci in range(NC_C):
                nc.tensor.matmul(
                    pv, xln1[ci][:, w * 196 + u * 98:
                                 w * 196 + (u + 1) * 98],
                    wv[ci][:, s * 512:(s + 1) * 512],
                    start=(ci == 0),
                    stop=(zero_bias and ci == NC_C - 1))
            if not zero_bias:
                nc.tensor.matmul(
                    pv, ones_row[0:1, :],
                    bvrow[0:1, s * 512:(s + 1) * 512],
                    start=False, stop=True)
            dst = vt[:, (w * 2 + u) * 512:(w * 2 + u + 1) * 512]
            if (w * 2 + u) % 2 == 0:
                nc.scalar.copy(out=dst, in_=pv)
            else:
                nc.vector.tensor_copy(out=dst, in_=pv)

        def emit_rel_group(grp, qrA, qrB):
            # 4 rel-pos idx per PSUM bank; one strided writeback per
            # quadrant instead of one per idx.
            i0 = 4 * grp
            ni = min(4, WS - i0)
            qvA = qrA.rearrange("p (w a b) -> p w a b", w=N_WIN, a=WS)
            qvB = qrB.rearrange("p (w a b) -> p w a b", w=N_WIN, a=WS)
            rp = p2ps.tile([128, 448], F32, tag="relps", name="relps",
                           bufs=2)
            for il in range(ni):
                idx = i0 + il
                cs = slice(il * 112, (il + 1) * 112)
                nc.tensor.matmul(
                    rp[64:78, cs], rhe_sb[0:64, idx * 14:(idx + 1) * 14],
                    qvA[0:64, :, idx, :], start=True, stop=True,
                    tile_position=(0, 64))
                nc.tensor.matmul(
                    rp[96:110, cs], rwe_sb[0:64, idx * 14:(idx + 1) * 14],
                    qvA[0:64, :, :, idx], start=True, stop=True,
                    tile_position=(0, 96))
                nc.tensor.matmul(
                    rp[0:14, cs], rhe_sb[64:128, idx * 14:(idx + 1) * 14],
                    qvB[64:128, :, idx, :], start=True, stop=True,
                    tile_position=(64, 0))
                nc.tensor.matmul(
                    rp[32:46, cs], rwe_sb[64:128, idx * 14:(idx + 1) * 14],
                    qvB[64:128, :, :, idx], start=True, stop=True,
                    tile_position=(64, 32))
            rp5 = rp.rearrange("p (i w b) -> p i w b", i=4, w=N_WIN)[:, 0:ni]
            nc.vector.tensor_copy(
                out=qvA[64:78, :, i0:i0 + ni, :],
                in_=rp5[64:78].transpose([0, 2, 1, 3]))
            nc.vector.tensor_copy(
                out=qvA[96:110, :, :, i0:i0 + ni],
                in_=rp5[96:110].transpose([0, 2, 3, 1]))
            nc.scalar.copy(
                out=qvB[0:14, :, i0:i0 + ni, :],
                in_=rp5[0:14].transpose([0, 2, 1, 3]))
            nc.scalar.copy(
                out=qvB[32:46, :, :, i0:i0 + ni],
                in_=rp5[32:46].transpose([0, 2, 3, 1]))

        def emit_scores(hp, w, qrA, qrB, kA, kB):
            # one [98,392] score tile per head (both k-halves share a PSUM
            # bank) -> single exp per head
            base = w * 196
            pts = []
            for head in range(2):
                st = p2ps.tile([98, 392], F32, tag="stps", name="stps",
                               bufs=3)
                for u in range(2):
                    if head == 0:
                        nc.tensor.matmul(
                            st[:, u * 196:(u + 1) * 196],
                            kA[0:110, base + u * 98:base + (u + 1) * 98],
                            qrA[0:110, base:base + 196],
                            start=True, stop=True, tile_position=(0, 0))
                    else:
                        nc.tensor.matmul(
                            st[:, u * 196:(u + 1) * 196],
                            kB[0:128, base + u * 98:base + (u + 1) * 98],
                            qrB[0:128, base:base + 196],
                            start=True, stop=True, tile_position=(0, 0))
                pt = p2.tile([98, 392], BF16, tag="pt", name="pt", bufs=6)
                nc.scalar.activation(out=pt, in_=st, func=AF.Exp)
                pts.append(pt)
            return pts

        def emit_pv(hp, w, pts, vt, smb, pob):
            ptA, ptB = pts
            pvt = p2ps.tile([128, 196], F32, tag="pvps", name="pvps", bufs=1)
            smt = p2ps.tile([128, TT], F32, tag="stps", name="smps", bufs=3)
            for u in range(2):
                vbase = (w * 2 + u) * 512 + (hp % 4) * 128
                nc.tensor.matmul(
                    pvt[0:64, :], vt[:, vbase:vbase + 64],
                    ptA[:, u * 196:(u + 1) * 196],
                    start=(u == 0), stop=(u == 1),
                    tile_position=(0, 0), skip_group_check=True)
                nc.tensor.matmul(
                    pvt[64:128, :], vt[:, vbase + 64:vbase + 128],
                    ptB[:, u * 196:(u + 1) * 196],
                    start=(u == 0), stop=(u == 1),
                    tile_position=(0, 64), skip_group_check=True)
            for u in range(2):
                nc.tensor.matmul(
                    smt[:, 0:196], ones_bf[0:98, :],
                    ptA[:, u * 196:(u + 1) * 196],
                    start=(u == 0), stop=(u == 1), skip_group_check=True)
            for u in range(2):
                nc.tensor.matmul(
                    smt[:, 196:392], ones_bf[0:98, :],
                    ptB[:, u * 196:(u + 1) * 196],
                    start=(u == 0), stop=(u == 1), skip_group_check=True)
            nc.vector.tensor_copy(out=smb[:, w * TT:(w + 1) * TT], in_=smt)
            nc.vector.tensor_copy(out=pob[:, w * 196:(w + 1) * 196], in_=pvt)

        def emit_den_muls(hp, smb, pob):
            # batched softmax denominators for all 8 windows of this hp:
            # 1/s = exp(-ln(s)); rs2b packs head A on parts 0:64, B on 64:128
            nc.scalar.activation(out=smb, in_=smb, func=AF.Ln)
            lt4 = smb.rearrange("p (w h q) -> p w h q", w=N_WIN, h=2)
            rs2b = p2.tile([128, 8 * 196], F32, tag="rs2b", name="rs2b",
                           bufs=1)
            rs4 = rs2b.rearrange("p (w q) -> p w q", w=N_WIN)
            nc.scalar.activation(out=rs4[0:64, :, :], in_=lt4[0:64, :, 0, :],
                                 func=AF.Exp, scale=-1.0)
            nc.scalar.activation(out=rs4[64:128, :, :],
                                 in_=lt4[64:128, :, 1, :],
                                 func=AF.Exp, scale=-1.0)
            for w in range(N_WIN):
                nc.vector.tensor_mul(
                    out=win_view(xattn[hp], w),
                    in0=pob[:, w * 196:(w + 1) * 196].rearrange(
                        "p (r c) -> p r c", r=WS),
                    in1=rs2b[:, w * 196:(w + 1) * 196].rearrange(
                        "p (r c) -> p r c", r=WS))

        def emit_windows(hp, rel_hp, vchains):
            qrA, qrB = qrelA[hp % 2], qrelB[hp % 2]
            kA, kB = kindA[hp % 2], kindB[hp % 2]
            vt = v_tiles[hp // 4]
            smb = p2.tile([128, 8 * TT], F32, tag="smb", name="smb", bufs=1)
            pob = p2.tile([128, 8 * 196], F32, tag="pob", name="pob", bufs=2)
            rel_steps = {1: 0, 3: 1, 5: 2, 7: 3} if rel_hp is not None else {}
            vchains = list(vchains)
            pts_q = {}
            for w in range(N_WIN + 2):
                if w < N_WIN:
                    pts_q[w] = emit_scores(hp, w, qrA, qrB, kA, kB)
                if w >= 2:
                    emit_pv(hp, w - 2, pts_q.pop(w - 2), vt, smb, pob)
                if w in rel_steps:
                    emit_rel_group(rel_steps[w],
                                   qrelA[rel_hp % 2], qrelB[rel_hp % 2])
                for _ in range(min(2, len(vchains))):
                    vs, vw, vu = vchains.pop(0)
                    emit_v_chain(vs, v_tiles[vs], vw, vu)
            emit_den_muls(hp, smb, pob)

        v_tiles = {}
        v_tiles[0] = v_alloc(0)
        for grp in range(4):
            emit_rel_group(grp, qrelA[0], qrelB[0])
        v_sched = {0: [(0, w, u) for w in range(N_WIN) for u in (0, 1)],
                   1: [(1, w, u) for w in (0, 1, 2) for u in (0, 1)],
                   2: [(1, w, u) for w in (3, 4, 5) for u in (0, 1)],
                   3: [(1, w, u) for w in (6, 7) for u in (0, 1)]}
        for hp in range(8):
            if hp + 1 < 8:
                emit_qk(hp + 1)
                emit_qk(9 + hp)
            if hp == 1:
                v_tiles[1] = v_alloc(1)
            emit_windows(hp, rel_hp=hp + 1 if hp + 1 < 8 else None,
                         vchains=v_sched.get(hp, []))
        dump("d_xattn", xattn)
        close(p2ps, pqkps, p2, pvv, pqk, pwq)

        # ---------------- Phase 3: proj + residual ----------------
        px2 = pool("px2", side="right")                     # x2, lives to the end
        p3 = pool("p3", bufs=2)
        p3ps = pspool("p3ps", bufs=2)
        x2 = []
        for ot in range(NC_C):
            x2.append(px2.tile([128, T], BF16,
                               tag=f"x2_{ot}", name=f"x2_{ot}"))
        wp = []
        for ci in range(NC_C):
            t = p3.tile([128, C], BF16, tag=f"wproj{ci}", name=f"wproj{ci}",
                        bufs=1)
            nc.sync.dma_start(out=t, in_=projwT[ci * 128:(ci + 1) * 128, :])
            wp.append(t)
        for tt in range(NTT):
            for ot in range(NC_C):
                pst = p3ps.tile([128, TT], F32, tag="projps", name="projps", bufs=3)
                for ci in range(NC_C):
                    nc.tensor.matmul(
                        pst, wp[ci][:, ot * 128:(ot + 1) * 128],
                        xattn[ci][:, tt * TT:(tt + 1) * TT],
                        start=(ci == 0), stop=(ci == NC_C - 1))
                tmp = p3.tile([128, TT], F32, tag="projtmp", name="projtmp")
                nc.scalar.activation(out=tmp, in_=pst, func=AF.Identity,
                                     bias=projb_sb[:, ot:ot + 1])
                xre = p3.tile([128, TT], BF16, tag="xre", name="xre")
                nc.sync.dma_start(
                    out=xre,
                    in_=xT[ot * 128:(ot + 1) * 128, tt * TT:(tt + 1) * TT])
                nc.vector.tensor_add(
                    out=x2[ot][:, tt * TT:(tt + 1) * TT], in0=tmp, in1=xre)
        dump("d_x2", x2)
        close(p3ps, p3, pc, pa)

        # ---------------- Phase 4: LN2 + MLP ----------------
        p4x = pool("p4x")                     # xln2
        p4w = pool("p4w", bufs=2)             # fc weight rings (DMA early)
        p4a = pool("p4a", bufs=2)
        p4aps = pspool("p4aps", bufs=2)

        def load_w1(og):
            w1 = []
            for ci in range(NC_C):
                t = p4w.tile([128, 512], BF16, tag=f"w1_{ci}",
                             name=f"w1_{ci}", bufs=2)
                nc.sync.dma_start(
                    out=t, in_=fc1wT[ci * 128:(ci + 1) * 128,
                                     og * 512:(og + 1) * 512])
                w1.append(t)
            return w1

        w1_pre = {0: load_w1(0), 1: load_w1(1)}
        xln2 = []
        for ci in range(NC_C):
            xln2.append(p4x.tile([128, T], BF16, tag=f"xln2_{ci}",
                                 name=f"xln2_{ci}"))

        def apply_ln2(sl, pl):
            for ci in range(NC_C):
                tmp = pl.tile([128, 392], F32, tag="lntmp", name="lntmp",
                              bufs=3)
                nc.vector.tensor_sub(out=tmp, in0=x2[ci][:, sl],
                                     in1=mb[:, sl])
                nc.vector.tensor_mul(out=xln2[ci][:, sl], in0=tmp,
                                     in1=rsb[:, sl])

        # stats for all chunks, but apply only chunks 0/1 now: fc1's tp=0
        # pass needs just those, so chunks 2/3 apply under the fc1 og loop
        for tt in range(NTT):
            sl = slice(tt * TT, (tt + 1) * TT)
            ln_chunk(nc, p4a, p4aps, x2, sl, mb, rsb, 1.0 / C, eps5)
            if tt < 2:
                apply_ln2(sl, p4a)
        dump("d_xln2", xln2)
        close(p4aps, p4a)

        p4 = pool("p4", bufs=2)
        p4ps = pspool("p4ps", bufs=2)
        hbuf = p4.tile([128, 32 * 784], BF16, tag="hbuf", name="hbuf", bufs=1)
        for tp in range(2):
            for og in range(8):
                w1 = (w1_pre[og] if (tp == 0 and og in w1_pre)
                      else load_w1(og))
                for tl in range(2):
                    for otl in range(4):
                        ot = og * 4 + otl
                        t0 = tp * 784 + tl * TT
                        pst = p4ps.tile([128, TT], F32, tag="fc1ps",
                                        name="fc1ps", bufs=3)
                        for ci in range(NC_C):
                            nc.tensor.matmul(
                                pst, w1[ci][:, otl * 128:(otl + 1) * 128],
                                xln2[ci][:, t0:t0 + TT],
                                start=(ci == 0), stop=(ci == NC_C - 1))
                        nc.scalar.activation(
                            out=hbuf[:, ot * 784 + tl * TT:
                                     ot * 784 + (tl + 1) * TT],
                            in_=pst, func=AF.Gelu,
                            bias=fc1b_sb[:, ot:ot + 1])
                if tp == 0 and og < 2:
                    apply_ln2(slice((2 + og) * TT, (3 + og) * TT), p4w)
            for og2 in range(4):
                w2 = []
                for ki in range(32):
                    t = p4w.tile([128, 256], BF16, tag=f"w2_{ki}",
                                 name=f"w2_{ki}", bufs=2)
                    nc.sync.dma_start(
                        out=t, in_=fc2wT[ki * 128:(ki + 1) * 128,
                                         og2 * 256:(og2 + 1) * 256])
                    w2.append(t)
                for otl in range(2):
                    ot = og2 * 2 + otl
                    for tl in range(2):
                        t0 = tp * 784 + tl * TT
                        pst = p4ps.tile([128, TT], F32, tag="fc2ps",
                                        name="fc2ps", bufs=3)
                        for ki in range(32):
                            nc.tensor.matmul(
                                pst, w2[ki][:, otl * 128:(otl + 1) * 128],
                                hbuf[:, ki * 784 + tl * TT:
                                     ki * 784 + (tl + 1) * TT],
                                start=(ki == 0), stop=(ki == 31))
                        tmp = p4.tile([128, TT], F32, tag="fc2tmp",
                                      name="fc2tmp")
                        nc.scalar.activation(
                            out=tmp, in_=pst, func=AF.Identity,
                            bias=fc2b_sb[:, ot:ot + 1])
                        nc.vector.tensor_add(
                            out=x2[ot][:, t0:t0 + TT],
                            in0=tmp, in1=x2[ot][:, t0:t0 + TT])
        if debug:
            for ki in range(32):
                nc.sync.dma_start(out=dbg["d_h"][ki * 128:(ki + 1) * 128, :],
                                  in_=hbuf[:, ki * 784:(ki + 1) * 784])
        dump("d_x3", x2)
        close(p4ps, p4, p4w, p4x)

        # ---------------- Phase 5: ResBottleneckBlock ----------------
        pcw = pool("pcw")                     # all conv weights, DMA'd early
        wc1 = []
        for ci in range(NC_C):
            t = pcw.tile([128, 512], BF16, tag=f"wc1_{ci}", name=f"wc1_{ci}")
            nc.sync.dma_start(out=t, in_=c1wT[ci * 128:(ci + 1) * 128, :])
            wc1.append(t)
        wc2 = {}
        for tap in range(9):
            for ci in range(4):
                t = pcw.tile([128, 512], BF16, tag=f"wc2_{tap}_{ci}",
                             name=f"wc2_{tap}_{ci}")
                nc.sync.dma_start(
                    out=t, in_=c2wT[tap, ci * 128:(ci + 1) * 128, :])
                wc2[(tap, ci)] = t
        wc3 = []
        for ci in range(4):
            t = pcw.tile([128, C], BF16, tag=f"wc3_{ci}", name=f"wc3_{ci}")
            nc.sync.dma_start(out=t, in_=c3wT[ci * 128:(ci + 1) * 128, :])
            wc3.append(t)
        pr1 = pool("pr1")                     # r1pad, lives 5a..5b
        r1pad = []
        for ci in range(4):
            t = pr1.tile([128, BS * 900 + 2], BF16, tag=f"r1pad{ci}",
                         name=f"r1pad{ci}")
            nc.gpsimd.memset(t, 0.0)
            r1pad.append(t)
        p5a = pool("p5a", bufs=2)
        p5aps = pspool("p5aps", bufs=2)
        c1s = [p5a.tile([128, T], BF16, tag=f"c1s{ot}",
                        name=f"c1s{ot}", bufs=1) for ot in range(4)]
        def apply_n1(sl):
            tt = sl.start // TT
            i, wh = tt // 2, tt % 2
            for ci in range(4):
                pv = r1pad[ci][:, 0:1800].rearrange(
                    "p (i y x) -> p i y x", i=BS, y=30)
                tmp = p5a.tile([128, TT], F32, tag="c1tmp", name="c1tmp",
                               bufs=3)
                nc.vector.tensor_sub(out=tmp, in0=c1s[ci][:, sl],
                                     in1=mb[:, sl])
                nc.vector.tensor_mul(out=tmp, in0=tmp, in1=rsb[:, sl])
                nc.scalar.activation(
                    out=pv[:, i, 1 + 14 * wh:15 + 14 * wh, 1:29],
                    in_=tmp, func=AF.Gelu,
                    bias=n1b_sb[:, ci:ci + 1], scale=n1w_sb[:, ci:ci + 1])

        # stats/apply trail the conv matmuls by one chunk so the PE never
        # waits on the copy->square chain
        for tt in range(NTT + 1):
            if tt < NTT:
                sl = slice(tt * TT, (tt + 1) * TT)
                for ot in range(4):
                    pst = p5aps.tile([128, TT], F32, tag="c1ps", name="c1ps",
                                     bufs=3)
                    for ci in range(NC_C):
                        nc.tensor.matmul(
                            pst, wc1[ci][:, ot * 128:(ot + 1) * 128],
                            x2[ci][:, sl],
                            start=(ci == 0), stop=(ci == NC_C - 1))
                    nc.vector.tensor_copy(out=c1s[ot][:, sl], in_=pst)
            if tt >= 1:
                sl = slice((tt - 1) * TT, tt * TT)
                ln_chunk(nc, p5a, p5aps, c1s, sl, mb, rsb, 1.0 / 512, eps6)
                apply_n1(sl)
        dump("d_c1", c1s)
        dump("d_r1p", r1pad)
        close(p5aps, p5a)

        pr2 = pool("pr2", side="right")                     # r2, lives 5b..5c
        p5b = pool("p5b", bufs=2)
        p5bps = pspool("p5bps", bufs=2)
        c2s = [p5b.tile([128, T], BF16, tag=f"c2s{ot}",
                        name=f"c2s{ot}", bufs=1) for ot in range(4)]
        def conv2_chunk(tt):
            for ot in range(4):
                i, wh = tt // 2, tt % 2
                pst = p5bps.tile([128, 420], F32, tag="c2ps", name="c2ps", bufs=3)
                n9 = 0
                for tap in range(9):
                    dy, dx = tap // 3, tap % 3
                    for ci in range(4):
                        st0 = i * 900 + (14 * wh + dy) * 30 + dx
                        nc.tensor.matmul(
                            pst, wc2[(tap, ci)][:, ot * 128:(ot + 1) * 128],
                            r1pad[ci][:, st0:st0 + 420],
                            start=(n9 == 0), stop=(n9 == 35))
                        n9 += 1
                nc.scalar.copy(
                    out=c2s[ot][:, tt * TT:(tt + 1) * TT].rearrange(
                        "p (h x) -> p h x", h=WS),
                    in_=pst.rearrange("p (h x) -> p h x", h=WS)[:, :, 0:28])
        r2 = []
        for ci in range(4):
            r2.append(pr2.tile([128, T], BF16, tag=f"r2_{ci}",
                               name=f"r2_{ci}"))

        def apply_n2(sl):
            for ci in range(4):
                tmp = p5b.tile([128, TT], F32, tag="c2tmp", name="c2tmp",
                               bufs=3)
                nc.vector.tensor_sub(out=tmp, in0=c2s[ci][:, sl],
                                     in1=mb[:, sl])
                nc.vector.tensor_mul(out=tmp, in0=tmp, in1=rsb[:, sl])
                nc.scalar.activation(
                    out=r2[ci][:, sl], in_=tmp, func=AF.Gelu,
                    bias=n2b_sb[:, ci:ci + 1], scale=n2w_sb[:, ci:ci + 1])

        for tt in range(NTT + 1):
            if tt < NTT:
                conv2_chunk(tt)
            if tt >= 1:
                sl = slice((tt - 1) * TT, tt * TT)
                ln_chunk(nc, p5b, p5bps, c2s, sl, mb, rsb, 1.0 / 512, eps6)
                apply_n2(sl)
        dump("d_c2", c2s)
        close(p5bps, p5b, pr1)

        p5c = pool("p5c", bufs=2)
        p5cps = pspool("p5cps", bufs=2)
        c3s = [p5c.tile([128, T], BF16, tag=f"c3s{ot}",
                        name=f"c3s{ot}", bufs=1) for ot in range(NC_C)]
        def apply_n3(s):
            # spread across DVE (sub/mul/scale), gpsimd (residual)
            for ot in range(NC_C):
                tmp = p5c.tile([128, s.stop - s.start], F32, tag="ytmp",
                               name="ytmp", bufs=3)
                nc.vector.tensor_sub(out=tmp, in0=c3s[ot][:, s], in1=mb[:, s])
                nc.vector.tensor_mul(out=tmp, in0=tmp, in1=rsb[:, s])
                if zero_bias:
                    nc.vector.tensor_scalar_mul(out=tmp, in0=tmp,
                                                scalar1=n3w_sb[:, ot:ot + 1])
                else:
                    nc.scalar.activation(
                        out=tmp, in_=tmp, func=AF.Identity,
                        scale=n3w_sb[:, ot:ot + 1], bias=n3b_sb[:, ot:ot + 1])
                yt = p5c.tile([128, s.stop - s.start], F32, tag="yt",
                              name="yt", bufs=3)
                nc.gpsimd.tensor_add(out=yt, in0=tmp, in1=x2[ot][:, s])
                nc.sync.dma_start(out=yT[ot * 128:(ot + 1) * 128, s], in_=yt)

        NSC = 8                 # 196-col sub-chunks: short epilogue chains
        for tt in range(NSC + 1):
            if tt < NSC:
                sl = slice(tt * 196, (tt + 1) * 196)
                for ot in range(NC_C):
                    pst = p5cps.tile([128, 196], F32, tag="c3ps", name="c3ps",
                                     bufs=3)
                    for ci in range(4):
                        nc.tensor.matmul(
                            pst, wc3[ci][:, ot * 128:(ot + 1) * 128],
                            r2[ci][:, sl],
                            start=(ci == 0), stop=(ci == 3))
                    nc.scalar.copy(out=c3s[ot][:, sl], in_=pst)
            if tt >= 1:
                sl = slice((tt - 1) * 196, tt * 196)
                ln_chunk(nc, p5c, p5cps, c3s, sl, mb, rsb, 1.0 / C, eps6)
                apply_n3(sl)
        dump("d_c3", c3s)
        close(p5cps, p5c, pcw, pr2, px2, g)

    return nc


# ---------------------------------------------------------------------------
# Host side
# ---------------------------------------------------------------------------
_program_cache = {}


def _get_program(zero_bias=False):
    key = ("nc", zero_bias)
    if key not in _program_cache:
        _program_cache[key] = build_program(zero_bias=zero_bias)
    return _program_cache[key]


def _bf(x):
    return np.ascontiguousarray(x).astype(ml_dtypes.bfloat16)


def prep_inputs(inputs):
    """Build the per-core input maps (host-side sharding + weight prep)."""
    f = {k: np.asarray(v, dtype=np.float32) for k, v in inputs.items()}
    scale = HD ** -0.5

    qkv_w = f["qkv_w"].copy()          # [3C, C]
    qkv_b = f["qkv_b"].copy()          # [3C]
    qkv_w[:C] *= scale                 # fold 1/sqrt(hd) into q
    qkv_b[:C] *= scale
    # fold ln1 affine into qkv
    qkv_wT = (qkv_w * f["ln1_w"][None, :]).T.copy()      # [C, 3C]
    qkv_b_eff = qkv_b + qkv_w @ f["ln1_b"]
    # fold ln2 affine into fc1
    fc1_wT = (f["fc1_w"] * f["ln2_w"][None, :]).T.copy()  # [C, MLP]
    fc1_b_eff = f["fc1_b"] + f["fc1_w"] @ f["ln2_b"]

    proj_wT = f["proj_w"].T.copy()
    fc2_wT = f["fc2_w"].T.copy()
    c1_wT = f["conv1_w"][:, :, 0, 0].T.copy()            # [C, C/2]
    c2 = f["conv2_w"]                                    # [O, I, 3, 3]
    c2_wT = np.ascontiguousarray(
        c2.transpose(2, 3, 1, 0).reshape(9, C // 2, C // 2))
    c3_wT = f["conv3_w"][:, :, 0, 0].T.copy()            # [C/2, C]

    # rel pos tables: rhe[p, qh*14+kh] = 8*rel_pos_h[qh-kh+13, p%64]
    rh8 = 8.0 * f["rel_pos_h"]                           # [27, 64]
    rw8 = 8.0 * f["rel_pos_w"]
    qh_i, kh_i = np.meshgrid(np.arange(WS), np.arange(WS), indexing="ij")
    idx = qh_i - kh_i + WS - 1                           # [qh, kh]
    rhe = rh8[idx]                                       # [qh, kh, 64]
    rwe = rw8[idx]
    rhe_t = np.zeros((128, 196), np.float32)
    rwe_t = np.zeros((128, 196), np.float32)
    rhe_flat = rhe.transpose(2, 0, 1).reshape(64, 196)   # [c, qh*14+kh]
    rwe_flat = rwe.transpose(2, 0, 1).reshape(64, 196)
    rhe_t[0:64] = rhe_flat
    rhe_t[64:128] = rhe_flat
    rwe_t[0:64] = rwe_flat
    rwe_t[64:128] = rwe_flat

    # indicators vs rel rows: A-kh 0:14, B-kh 32:46, A-kw 64:78, B-kw 96:110
    indA = np.zeros((128, 196), np.float32)
    indB = np.zeros((128, 196), np.float32)
    kt = np.arange(196)
    for j in range(WS):
        indA[j, kt // 14 == j] = 1.0
        indA[64 + j, kt % 14 == j] = 1.0
        indB[32 + j, kt // 14 == j] = 1.0
        indB[96 + j, kt % 14 == j] = 1.0
    # device-side kind/qrel init templates (rows 64:110 of kindA, 0:64 of
    # kindB, plus an 18-row zero slab for the qrel pad rows)
    kindA_init = np.zeros((46, T), np.float32)
    kindB_init = np.zeros((64, T), np.float32)
    for w in range(8):
        cs = slice(w * 196, (w + 1) * 196)
        kindA_init[0:14, cs] = indA[0:14]
        kindA_init[32:46, cs] = indA[64:78]
        kindB_init[0:14, cs] = indB[32:46]
        kindB_init[32:46, cs] = indB[96:110]

    common = {
        "qkvwT": _bf(qkv_wT),
        "qkvb": qkv_b_eff.astype(np.float32),
        "qkvbv": _bf(qkv_b_eff[2 * C:][None, :]),
        "projwT": _bf(proj_wT),
        "projb": f["proj_b"],
        "fc1wT": _bf(fc1_wT),
        "fc1b": fc1_b_eff.astype(np.float32),
        "fc2wT": _bf(fc2_wT),
        "fc2b": f["fc2_b"],
        "c1wT": _bf(c1_wT),
        "c2wT": _bf(c2_wT),
        "c3wT": _bf(c3_wT),
        "n1w": f["n1_w"], "n1b": f["n1_b"],
        "n2w": f["n2_w"], "n2b": f["n2_b"],
        "n3w": f["n3_w"], "n3b": f["n3_b"],
        "rhe": _bf(rhe_t), "rwe": _bf(rwe_t),
        "kindAi": _bf(kindA_init), "kindBi": _bf(kindB_init),
        "zpad": np.zeros((18, T), ml_dtypes.bfloat16),
    }
    x = f["x"]                                           # [B, 28, 28, C]
    in_maps = []
    for core in range(NCORES):
        xs = x[core * BS:(core + 1) * BS].reshape(T, C).T  # [C, T]
        m = dict(common)
        m["xT"] = _bf(xs)
        in_maps.append(m)
    return in_maps


def run(inputs, trace=False):
    from concourse.bass_utils import run_bass_kernel_spmd

    in_maps = prep_inputs(inputs)
    zb = bool(np.all(in_maps[0]["qkvb"] == 0.0)
              and np.all(in_maps[0]["n3b"] == 0.0))
    nc = _get_program(zero_bias=zb)
    res = run_bass_kernel_spmd(nc, in_maps, core_ids=list(range(NCORES)),
                               trace=trace)
    outs = []
    for core in range(NCORES):
        yt = res.results[core]["yT"]                     # [C, T]
        outs.append(yt.T.reshape(BS, HH, WW, C))
    y = np.concatenate(outs, axis=0).astype(np.float32)
    return y, res


def kernel(**inputs):
    y, _ = run(inputs, trace=False)
    return y



# revision 91
# speedup vs baseline: 1.0103x; 1.0103x over previous
"""Trainium2 Bass kernel for nn_Extractor_Processor_75368086110414.

Windowed-attention transformer block (ViTDet-style) + ResBottleneckBlock,
data-parallel over batch across 8 NeuronCores (2 images per core).

Device layout: activations live TRANSPOSED in SBUF as [feature, token] with
features on partitions, tokens (row-major per image) on the free dim. All
matmuls put the contraction dim on partitions (W^T tiles pre-transposed on
host). LayerNorm statistics (over channels = partitions) are computed with
ones-matmuls on the tensor engine; per-token stats are broadcast back across
partitions via a DMA round-trip through DRAM scratch.

Attention avoids materializing any transposes: S^T = k q^T (+ decomposed
rel-pos bias injected through an indicator matmul accumulated in PSUM),
exp on ACT, out^T = (P^T)^T-free col-tiled PV matmuls, and softmax
denominators from ones-matmul column sums of P^T.
"""

import json
import sys
import types

import numpy as np
import ml_dtypes

import concourse.bass as bass
import concourse.tile as tile
from concourse import mybir
from concourse.vector_clock import ScopedClock
from concourse.masks import make_identity

F32 = mybir.dt.float32
BF16 = mybir.dt.bfloat16
AF = mybir.ActivationFunctionType
OP = mybir.AluOpType

# ---------------------------------------------------------------------------
# Patch 1: the pinned walrus rejects >1 sync wait per instruction. Split the
# kernel-tail drain's waits across a chain of drains, and post-process the
# BIR JSON to peel extra waits off any instruction onto injected NoOps.
# ---------------------------------------------------------------------------
MAX_WAITS = 1
_patched = False


def _drain_and_barrier(self, tick_clock, wait_clock):
    nc = self.nc
    drain_inst = nc.sync.drain()
    wait_clock.add_sem_waits(
        drain_inst.ins, ScopedClock({None: tick_clock.global_clock})
    )
    waits = list(drain_inst.ins.sync_info.on_wait)
    if len(waits) > MAX_WAITS:
        drain_inst.ins.sync_info = mybir.SyncInfo(
            on_wait=waits[:MAX_WAITS], on_update=[]
        )
        rest = waits[MAX_WAITS:]
        for i in range(0, len(rest), MAX_WAITS):
            extra = nc.sync.drain()
            extra.ins.sync_info = mybir.SyncInfo(
                on_wait=rest[i : i + MAX_WAITS], on_update=[]
            )
    nc.all_engine_barrier()
    assert self.sems is not None
    popped = nc._tile_sem_poison_stack.pop()
    assert popped is self._sem_poison
    nc.clear_and_free_semaphores(list(self.sems.allocated().values()))
    nc.all_engine_barrier()


def _split_waits_json(data: bytes) -> bytes:
    bj = json.loads(data)
    counter = [0]
    changed = False
    for fn in bj.get("functions", []):
        for bb in fn.get("blocks", []):
            insts = bb.get("instructions")
            if not insts:
                continue
            out = []
            for inst in insts:
                si = inst.get("sync_info")
                waits = si.get("on_wait") if si else None
                if waits and len(waits) > MAX_WAITS:
                    keep = waits[-MAX_WAITS:]
                    rest = waits[:-MAX_WAITS]
                    for i in range(0, len(rest), MAX_WAITS):
                        counter[0] += 1
                        out.append({
                            "debug": inst.get("debug"),
                            "engine": inst["engine"],
                            "ins": [],
                            "name": f"I-ws{counter[0]}",
                            "opcode": "NoOp",
                            "outs": [],
                            "sync_info": {
                                "on_wait": rest[i : i + MAX_WAITS],
                                "on_update": [],
                            },
                        })
                    si["on_wait"] = keep
                    changed = True
                out.append(inst)
            bb["instructions"] = out
    if not changed:
        return data
    return json.dumps(bj).encode()


def _apply_patches():
    global _patched
    if _patched:
        return
    _patched = True
    tile.TileContext._drain_and_barrier = _drain_and_barrier
    orig = bass.Bass.to_json_bytes
    bass.Bass.to_json_bytes = lambda self, *a, **kw: _split_waits_json(
        orig(self, *a, **kw)
    )
    # Patch 2: the agent image's antenv lacks axon_hooks; register a shim so
    # run_bass_kernel_spmd(trace=True) can find the NTFF profile hook.
    if "antenv.axon_hooks" not in sys.modules:
        try:
            from trn_agent_boot.trn_boot import _ntff_profile_via_ctypes

            hook = _ntff_profile_via_ctypes("/opt/axon/libaxon_pjrt.so")
        except Exception:
            hook = None
        mod = types.ModuleType("antenv.axon_hooks")
        mod.get_axon_ntff_profile_hook = lambda: hook
        mod.set_axon_ntff_profile_hook = lambda h: None
        sys.modules["antenv.axon_hooks"] = mod


_apply_patches()

# ---------------------------------------------------------------------------
# Problem geometry (hardcoded per spec)
# ---------------------------------------------------------------------------
C = 1024
NH = 16
HD = 64
WS = 14
MLP = 4096
B, HH, WW = 16, 28, 28
NCORES = 8
BS = B // NCORES          # images per core
T = BS * HH * WW          # 1568 tokens per core
TT = 392                  # token tile (one 14-row window-band of one image)
NTT = T // TT             # 4
NC_C = C // 128           # 8 c-tiles
N_WIN = BS * 4            # 8 windows per core
N_CHUNK = 16              # LN stat chunks of 98 tokens (window halves)


def _win_base(w):
    i, wh, ww = w // 4, (w // 2) % 2, w % 2
    return i * 784 + wh * 392 + ww * 14, (i, wh, ww)


def _r4(t):
    # [128, T] -> [p, i, wh, r, w28]
    return t.rearrange("p (i wh r w) -> p i wh r w", i=BS, wh=2, r=WS)


def win_view(t, w):
    i, wh, ww = w // 4, (w // 2) % 2, w % 2
    return _r4(t)[:, i, wh, :, ww * 14:(ww + 1) * 14]          # [p,14,14]


def half_view(t, w, u):
    i, wh, ww = w // 4, (w // 2) % 2, w % 2
    return _r4(t)[:, i, wh, u * 7:(u + 1) * 7, ww * 14:(ww + 1) * 14]  # [p,7,14]


def chunk_view(t, u):
    # flat contiguous 98-token chunk u (any token partition works for
    # per-token stats over channels)
    return t[:, u * 98:(u + 1) * 98]


def pair_view(t, i, wh, hf):
    # two chunks (ww=0,1) as [p, rr7, ww2, c14] but ordered (ww, rr, c) to
    # match a flat (ww-major) source stream
    r6 = t.rearrange("p (i wh hf rr ww c) -> p i wh hf rr ww c",
                     i=BS, wh=2, hf=2, rr=7, ww=2)
    return r6[:, i, wh, hf, :, :, :].transpose([0, 2, 1, 3])   # [p, ww2, rr7, c14]


# ---------------------------------------------------------------------------
# LayerNorm-over-partitions helper
# ---------------------------------------------------------------------------
def ln_chunk(nc, sbp, psp, src_tiles, sl, mb, rsb, inv_c, eps_tile):
    """Emit squares + ones-matmul stats + mean/rsqrt epilogue for one
    392-column chunk. Squares go to bf16 ring tiles so the sq-sum matmuls
    run at bf16 rate; the sum matmuls consume the source tiles directly."""
    ncti = len(src_tiles)
    w = sl.stop - sl.start
    F32R = mybir.dt.float32r
    ones_r = globals()["_ones_fr"].bitcast(F32R)
    ones_b = globals()["_ones_bf"]
    src_bf = src_tiles[0].dtype == BF16
    sqs = []
    for ci in range(ncti):
        sq = sbp.tile([128, w], BF16, tag=f"lnsq{ci}", name=f"lnsq{ci}",
                      bufs=2)
        nc.scalar.activation(out=sq, in_=src_tiles[ci][:, sl],
                             func=AF.Square)
        sqs.append(sq)
    ps_s = psp.tile([128, w], F32, tag="lnsum", name="lnsum", bufs=2)
    ps_q = psp.tile([128, w], F32, tag="lnsqp", name="lnsqp", bufs=2)
    for ci in range(ncti):
        nc.tensor.matmul(ps_s, ones_b if src_bf else ones_r,
                         src_tiles[ci][:, sl],
                         start=(ci == 0), stop=(ci == ncti - 1))
    for ci in range(ncti):
        nc.tensor.matmul(ps_q, ones_b, sqs[ci],
                         start=(ci == 0), stop=(ci == ncti - 1))
    nc.scalar.activation(out=mb[:, sl], in_=ps_s, func=AF.Copy,
                         scale=inv_c)
    msq = sbp.tile([128, w], F32, tag="lnmsq", name="lnmsq", bufs=2)
    nc.vector.tensor_mul(out=msq, in0=mb[:, sl], in1=mb[:, sl])
    sqm = sbp.tile([128, w], F32, tag="lnsqm", name="lnsqm", bufs=2)
    nc.scalar.activation(out=sqm, in_=ps_q, func=AF.Copy, scale=inv_c)
    ve = sbp.tile([128, w], F32, tag="lnve", name="lnve", bufs=2)
    nc.vector.tensor_sub(out=ve, in0=sqm, in1=msq)
    nc.scalar.activation(out=ve, in_=ve, func=AF.Ln, bias=eps_tile)
    nc.scalar.activation(out=rsb[:, sl], in_=ve, func=AF.Exp,
                         scale=-0.5)


def ln_stats(nc, pools, src_tiles, scratch, mb, rsb, inv_c, eps_tile, ident,
             apply_cb=None):
    sbp, psp = pools
    for t0 in range(0, T, 392):
        sl = slice(t0, t0 + 392)
        ln_chunk(nc, sbp, psp, src_tiles, sl, mb, rsb, inv_c, eps_tile)
        if apply_cb is not None:
            apply_cb(sl)


# ---------------------------------------------------------------------------
# Program builder
# ---------------------------------------------------------------------------
ones_f32 = None  # set inside build


def build_program(debug=False, zero_bias=False):
    global ones_f32
    nc = bass.Bass()

    def din(name, shape, dt=F32):
        return nc.declare_dram_parameter(name, shape, dt, isOutput=False)

    xT = din("xT", [C, T], BF16)
    qkvwT = din("qkvwT", [C, 3 * C], BF16)
    qkvb = din("qkvb", [3 * C])
    qkvbv = din("qkvbv", [1, C], BF16)
    projwT = din("projwT", [C, C], BF16)
    projb = din("projb", [C])
    fc1wT = din("fc1wT", [C, MLP], BF16)
    fc1b = din("fc1b", [MLP])
    fc2wT = din("fc2wT", [MLP, C], BF16)
    fc2b = din("fc2b", [C])
    c1wT = din("c1wT", [C, C // 2], BF16)
    c2wT = din("c2wT", [9, C // 2, C // 2], BF16)
    c3wT = din("c3wT", [C // 2, C], BF16)
    n1w = din("n1w", [C // 2])
    n1b = din("n1b", [C // 2])
    n2w = din("n2w", [C // 2])
    n2b = din("n2b", [C // 2])
    n3w = din("n3w", [C])
    n3b = din("n3b", [C])
    rhe = din("rhe", [128, 196], BF16)
    rwe = din("rwe", [128, 196], BF16)
    kindAi = din("kindAi", [46, T], BF16)
    kindBi = din("kindBi", [64, T], BF16)
    zpad = din("zpad", [18, T], BF16)
    yT = nc.declare_dram_parameter("yT", [C, T], F32, isOutput=True)
    dbg = {}
    if debug:
        for name, shape, dt in [
            ("d_xln1", [C, T], BF16), ("d_qk", [2 * C, T], BF16),
            ("d_v", [98, N_WIN * 2 * C], BF16), ("d_rel", [128, T], BF16),
            ("d_xattn", [C, T], BF16), ("d_x2", [C, T], F32),
            ("d_xln2", [C, T], BF16), ("d_h", [MLP, 784], BF16),
            ("d_x3", [C, T], F32),
            ("d_c1", [C // 2, T], F32), ("d_r1p", [C // 2, BS * 900 + 2], BF16),
            ("d_c2", [C // 2, T], F32), ("d_c3", [C, T], F32),
            ("d_mb", [128, T], F32), ("d_rsb", [128, T], F32),
        ]:
            dbg[name] = nc.declare_dram_parameter(name, shape, dt, isOutput=True)

    def dump(name, tiles, rows=128):
        if not debug:
            return
        d = dbg[name]
        for i, t in enumerate(tiles):
            nc.sync.dma_start(out=d[i * rows:(i + 1) * rows, :][0:rows, :],
                              in_=t[0:rows, :] if rows != 98 else t)


    scratch = nc.dram_tensor("scratch", [2 * 98 * N_CHUNK], F32)
    scratch2 = nc.dram_tensor("scratch2", [64 * 392], F32)

    _cms = {}

    def pool(name, bufs=1, side=None):
        cm = tc.tile_pool(name=name, bufs=bufs, side=side)
        p = cm.__enter__()
        _cms[id(p)] = cm
        return p

    def pspool(name, bufs=1):
        cm = tc.tile_pool(name=name, bufs=bufs, space="PSUM")
        p = cm.__enter__()
        _cms[id(p)] = cm
        return p

    def close(*pools_):
        for p in pools_:
            _cms.pop(id(p)).__exit__(None, None, None)

    with tile.TileContext(nc) as tc:
        g = pool("glob")
        global ones_f32
        ones_f32 = g.tile([128, 1], F32, tag="ones_f32", name="ones_f32")
        nc.vector.memset(ones_f32, 1.0)
        ones_bf = g.tile([128, 128], BF16, tag="ones_bf", name="ones_bf")
        nc.vector.memset(ones_bf, 1.0)
        ones_fr = g.tile([128, 128], F32, tag="ones_fr", name="ones_fr")
        nc.vector.memset(ones_fr, 1.0)
        globals()["_ones_fr"] = ones_fr
        globals()["_ones_bf"] = ones_bf
        eps5 = g.tile([128, 1], F32, tag="eps5", name="eps5")
        nc.vector.memset(eps5, 1e-5)
        eps6 = g.tile([128, 1], F32, tag="eps6", name="eps6")
        nc.vector.memset(eps6, 1e-6)
        ident = g.tile([128, 128], F32, tag="ident", name="ident")
        make_identity(nc, ident)

        pa = pool("pa")                       # xln1, lives through qkv
        pc = pool("pc")                       # xattn, lives thru proj
        pwq = pool("pwq")                     # qkv weights (DMA'd early)
        p1x = pool("p1x")                     # raw x, dies after apply
        p1 = pool("p1", bufs=2)
        pqkps = pspool("pqkps", bufs=1)       # qk/v chain ring (2 banks),
        p1ps = pspool("p1ps", bufs=2)         # coexists with LN1 psum
        F32R = mybir.dt.float32r
        xt = []
        for ci in range(NC_C):
            xt.append(p1x.tile([128, T], BF16, tag=f"xt{ci}", name=f"xt{ci}"))
        for half in range(2):
            sl = slice(half * 2 * TT, (half + 1) * 2 * TT)
            for ci in range(NC_C):
                nc.sync.dma_start(out=xt[ci][:, sl],
                                  in_=xT[ci * 128:(ci + 1) * 128, sl])
        # rel-pos tables first (tiny)
        rhe_sb = g.tile([128, 196], BF16, tag="rhe_sb", name="rhe_sb")
        nc.sync.dma_start(out=rhe_sb, in_=rhe[:, :])
        rwe_sb = g.tile([128, 196], BF16, tag="rwe_sb", name="rwe_sb")
        nc.sync.dma_start(out=rwe_sb, in_=rwe[:, :])
        # qkv weights next on the queue: quarters q0 (q heads 0-7) and q2
        # (k heads 0-7) first so qk(0) can start as soon as LN1 drains.
        wq = []
        for ci in range(NC_C):
            wq.append(pwq.tile([128, 2 * C], BF16, tag=f"wqk{ci}",
                               name=f"wqk{ci}"))
        wv = []
        for ci in range(NC_C):
            wv.append(pwq.tile([128, C], BF16, tag=f"wv{ci}", name=f"wv{ci}"))
        for quad in (0, 2):
            for ci in range(NC_C):
                nc.sync.dma_start(
                    out=wq[ci][:, quad * 512:(quad + 1) * 512],
                    in_=qkvwT[ci * 128:(ci + 1) * 128,
                              quad * 512:(quad + 1) * 512])

        def stage_bias(src, n, name):
            t = g.tile([128, n], F32, tag=name, name=name)
            nc.sync.dma_start(out=t, in_=src.rearrange("(o p) -> p o", p=128))
            return t

        qkvb_sb = stage_bias(qkvb, 24, "qkvb_sb")
        for ci in range(NC_C):
            nc.sync.dma_start(
                out=wv[ci], in_=qkvwT[ci * 128:(ci + 1) * 128, 2 * C:3 * C])
        for quad in (1, 3):
            for ci in range(NC_C):
                nc.sync.dma_start(
                    out=wq[ci][:, quad * 512:(quad + 1) * 512],
                    in_=qkvwT[ci * 128:(ci + 1) * 128,
                              quad * 512:(quad + 1) * 512])
        projb_sb = stage_bias(projb, 8, "projb_sb")

        # qrel/kind tiles; pad rows and indicator rows come straight from
        # host-prepared DRAM templates (zero engine time). qrel pad rows
        # must be zeroed: garbage bf16 can be Inf/NaN and 0 (stationary)
        # * Inf = NaN in the accumulation.
        qrelA = [pwq.tile([128, T], BF16, tag=f"qrelA{i}", name=f"qrelA{i}")
                 for i in range(2)]
        qrelB = [pwq.tile([128, T], BF16, tag=f"qrelB{i}", name=f"qrelB{i}")
                 for i in range(2)]
        kindA = [pwq.tile([128, T], BF16, tag=f"kindA{i}", name=f"kindA{i}")
                 for i in range(2)]
        kindB = [pwq.tile([128, T], BF16, tag=f"kindB{i}", name=f"kindB{i}")
                 for i in range(2)]
        for i in range(2):
            nc.sync.dma_start(out=kindA[i][64:110, :], in_=kindAi[:, :])
            nc.sync.dma_start(out=kindB[i][0:64, :], in_=kindBi[:, :])
            nc.sync.dma_start(out=qrelA[i][78:96, :], in_=zpad[:, :])
            nc.sync.dma_start(out=qrelB[i][14:32, :], in_=zpad[:, :])
            nc.sync.dma_start(out=qrelB[i][46:64, :], in_=zpad[:, :])
        fc1b_sb = stage_bias(fc1b, 32, "fc1b_sb")
        fc2b_sb = stage_bias(fc2b, 8, "fc2b_sb")
        n1w_sb = stage_bias(n1w, 4, "n1w_sb")
        n1b_sb = stage_bias(n1b, 4, "n1b_sb")
        n2w_sb = stage_bias(n2w, 4, "n2w_sb")
        n2b_sb = stage_bias(n2b, 4, "n2b_sb")
        n3w_sb = stage_bias(n3w, 8, "n3w_sb")
        n3b_sb = stage_bias(n3b, 8, "n3b_sb")

        mb = g.tile([128, T], F32, tag="mb", name="mb")
        rsb = g.tile([128, T], F32, tag="rsb", name="rsb")

        # ---------------- Phase 1: LN1 -> XLn1 (bf16), pipelined ----------
        # x arrives in per-chunk DMAs; stats + apply are emitted per 392-col
        # chunk so the first qk matmuls can start ~15us in.
        # xln1 is stored WINDOW-ORDERED: col = w*196 + r*14 + c
        xln1 = []
        for ci in range(NC_C):
            xln1.append(pa.tile([128, T], BF16, tag=f"xln1_{ci}",
                                name=f"xln1_{ci}"))

        def emit_qk(ot, tts=None, epi_dve=True):
            hp2 = ot % 8
            tA, tB = ((qrelA, qrelB) if ot < 8 else (kindA, kindB))
            for tt in (range(NTT) if tts is None else tts):
                s = slice(tt * TT, (tt + 1) * TT)
                pst = pqkps.tile([128, TT], F32, tag="bigps", name="qkps",
                                 bufs=2)
                for ci in range(NC_C):
                    nc.tensor.matmul(
                        pst, wq[ci][:, ot * 128:(ot + 1) * 128],
                        xln1[ci][:, s],
                        start=(ci == 0), stop=(ci == NC_C - 1))
                nc.scalar.activation(out=tA[hp2 % 2][0:64, s],
                                     in_=pst[0:64, :],
                                     func=AF.Identity,
                                     bias=qkvb_sb[0:64, ot:ot + 1])
                if zero_bias and epi_dve:
                    nc.vector.tensor_copy(out=tB[hp2 % 2][64:128, s],
                                          in_=pst[64:128, :])
                else:
                    nc.scalar.activation(out=tB[hp2 % 2][64:128, s],
                                         in_=pst[64:128, :],
                                         func=AF.Identity,
                                         bias=qkvb_sb[64:128, ot:ot + 1])

        for tt in range(NTT):
            sl = slice(tt * TT, (tt + 1) * TT)
            sqs = []
            for ci in range(NC_C):
                sq = p1.tile([128, TT], BF16, tag=f"lnsq{ci}",
                             name=f"lnsq{ci}", bufs=2)
                nc.scalar.activation(out=sq, in_=xt[ci][:, sl],
                                     func=AF.Square)
                sqs.append(sq)
            ps_s = p1ps.tile([128, TT], F32, tag="lnsum", name="lnsum", bufs=2)
            ps_q = p1ps.tile([128, TT], F32, tag="lnsqp", name="lnsqp", bufs=2)
            for ci in range(NC_C):
                nc.tensor.matmul(ps_s, ones_bf, xt[ci][:, sl],
                                 start=(ci == 0), stop=(ci == NC_C - 1))
            for ci in range(NC_C):
                nc.tensor.matmul(ps_q, ones_bf, sqs[ci],
                                 start=(ci == 0), stop=(ci == NC_C - 1))
            nc.scalar.activation(out=mb[:, sl], in_=ps_s, func=AF.Copy,
                                 scale=1.0 / C)
            msq = p1.tile([128, TT], F32, tag="lnmsq", name="lnmsq", bufs=2)
            nc.vector.tensor_mul(out=msq, in0=mb[:, sl], in1=mb[:, sl])
            sqm = p1.tile([128, TT], F32, tag="lnsqm", name="lnsqm", bufs=2)
            nc.scalar.activation(out=sqm, in_=ps_q, func=AF.Copy,
                                 scale=1.0 / C)
            ve = p1.tile([128, TT], F32, tag="lnve", name="lnve", bufs=2)
            nc.vector.tensor_sub(out=ve, in0=sqm, in1=msq)
            nc.scalar.activation(out=ve, in_=ve, func=AF.Ln, bias=eps5)
            nc.scalar.activation(out=rsb[:, sl], in_=ve, func=AF.Exp,
                                 scale=-0.5)
            # apply: row-major (r, ww, c) -> window-ordered (ww, r, c)
            for ci in range(NC_C):
                tmp = p1.tile([128, TT], F32, tag="lntmp", name="lntmp",
                              bufs=3)
                nc.vector.tensor_sub(out=tmp, in0=xt[ci][:, sl],
                                     in1=mb[:, sl])
                nc.vector.tensor_mul(
                    out=xln1[ci][:, sl].rearrange(
                        "p (w r c) -> p w r c", w=2, r=WS),
                    in0=tmp.rearrange("p (r w c) -> p w r c", w=2, r=WS),
                    in1=rsb[:, sl].rearrange("p (r w c) -> p w r c",
                                             w=2, r=WS))
            # fuse the first head-pair's qk chains chunk-by-chunk so their
            # epilogues interleave with the LN1 applies on DVE/ACT
            emit_qk(0, tts=[tt])
            emit_qk(8, tts=[tt])
        dump("d_xln1", xln1)
        dump("d_mb", [mb])
        dump("d_rsb", [rsb])
        close(p1ps, p1, p1x)

        # ------- Block 1: qkv + attention, software-pipelined -------
        # Phase hp emits: qk matmuls for hp+1, then the window loop for hp
        # with rel-pos writeback for hp+1 (and v chains for s=1 on phases
        # 1-3) interleaved so the PE stream stays dense.
        pqk = pool("pqk", side="right")       # rotating qk tiles
        pvv = pool("pvv", side="right")       # v slices
        p2 = pool("p2", bufs=2)               # rel/pts/den tmps
        p2ps = pspool("p2ps", bufs=1)
        bvrow = pwq.tile([1, C], BF16, tag="bvrow", name="bvrow")
        nc.sync.dma_start(out=bvrow, in_=qkvbv[:, :])
        ones_row = pwq.tile([1, 98], BF16, tag="ones_row", name="ones_row")
        nc.vector.memset(ones_row, 1.0)

        xattn = []
        for hp in range(8):
            xattn.append(pc.tile([128, T], BF16, tag=f"xattn{hp}",
                                 name=f"xattn{hp}"))

        # kqind merge: stationary kind = [k rows | ind rows], moving qrel =
        # [q rows | rel rows]; one K=110/128 matmul replaces kq+ind pairs.
        #   qrelA/kindA rows: 0:64 q/k, 64:78 kh, 78:96 zero, 96:110 kw
        #   qrelB/kindB rows: 0:14 kh, 32:46 kw, 46:64 zero, 64:128 q/k
        def v_alloc(s):
            return pvv.tile([98, 16 * 512], BF16, tag=f"v{s}", name=f"v{s}")

        def emit_v_chain(s, vt, w, u):
            pv = pqkps.tile([98, 512], F32, tag="bigps", name="vps",
                            bufs=2)
            for ci in range(NC_C):
                nc.tensor.matmul(
                    pv, xln1[ci][:, w * 196 + u * 98:
                                 w * 196 + (u + 1) * 98],
                    wv[ci][:, s * 512:(s + 1) * 512],
                    start=(ci == 0),
                    stop=(zero_bias and ci == NC_C - 1))
            if not zero_bias:
                nc.tensor.matmul(
                    pv, ones_row[0:1, :],
                    bvrow[0:1, s * 512:(s + 1) * 512],
                    start=False, stop=True)
            dst = vt[:, (w * 2 + u) * 512:(w * 2 + u + 1) * 512]
            if (w * 2 + u) % 2 == 0:
                nc.scalar.copy(out=dst, in_=pv)
            else:
                nc.vector.tensor_copy(out=dst, in_=pv)

        def emit_rel_group(grp, qrA, qrB):
            # 4 rel-pos idx per PSUM bank; one strided writeback per
            # quadrant instead of one per idx.
            i0 = 4 * grp
            ni = min(4, WS - i0)
            qvA = qrA.rearrange("p (w a b) -> p w a b", w=N_WIN, a=WS)
            qvB = qrB.rearrange("p (w a b) -> p w a b", w=N_WIN, a=WS)
            rp = p2ps.tile([128, 448], F32, tag="relps", name="relps",
                           bufs=2)
            for il in range(ni):
                idx = i0 + il
                cs = slice(il * 112, (il + 1) * 112)
                nc.tensor.matmul(
                    rp[64:78, cs], rhe_sb[0:64, idx * 14:(idx + 1) * 14],
                    qvA[0:64, :, idx, :], start=True, stop=True,
                    tile_position=(0, 64))
                nc.tensor.matmul(
                    rp[96:110, cs], rwe_sb[0:64, idx * 14:(idx + 1) * 14],
                    qvA[0:64, :, :, idx], start=True, stop=True,
                    tile_position=(0, 96))
                nc.tensor.matmul(
                    rp[0:14, cs], rhe_sb[64:128, idx * 14:(idx + 1) * 14],
                    qvB[64:128, :, idx, :], start=True, stop=True,
                    tile_position=(64, 0))
                nc.tensor.matmul(
                    rp[32:46, cs], rwe_sb[64:128, idx * 14:(idx + 1) * 14],
                    qvB[64:128, :, :, idx], start=True, stop=True,
                    tile_position=(64, 32))
            rp5 = rp.rearrange("p (i w b) -> p i w b", i=4, w=N_WIN)[:, 0:ni]
            nc.vector.tensor_copy(
                out=qvA[64:78, :, i0:i0 + ni, :],
                in_=rp5[64:78].transpose([0, 2, 1, 3]))
            nc.vector.tensor_copy(
                out=qvA[96:110, :, :, i0:i0 + ni],
                in_=rp5[96:110].transpose([0, 2, 3, 1]))
            nc.scalar.copy(
                out=qvB[0:14, :, i0:i0 + ni, :],
                in_=rp5[0:14].transpose([0, 2, 1, 3]))
            nc.scalar.copy(
                out=qvB[32:46, :, :, i0:i0 + ni],
                in_=rp5[32:46].transpose([0, 2, 3, 1]))

        def emit_scores(hp, w, qrA, qrB, kA, kB):
            # one [98,392] score tile per head (both k-halves share a PSUM
            # bank) -> single exp per head
            base = w * 196
            pts = []
            for head in range(2):
                st = p2ps.tile([98, 392], F32, tag="stps", name="stps",
                               bufs=3)
                for u in range(2):
                    if head == 0:
                        nc.tensor.matmul(
                            st[:, u * 196:(u + 1) * 196],
                            kA[0:110, base + u * 98:base + (u + 1) * 98],
                            qrA[0:110, base:base + 196],
                            start=True, stop=True, tile_position=(0, 0))
                    else:
                        nc.tensor.matmul(
                            st[:, u * 196:(u + 1) * 196],
                            kB[0:128, base + u * 98:base + (u + 1) * 98],
                            qrB[0:128, base:base + 196],
                            start=True, stop=True, tile_position=(0, 0))
                pt = p2.tile([98, 392], BF16, tag="pt", name="pt", bufs=6)
                nc.scalar.activation(out=pt, in_=st, func=AF.Exp)
                pts.append(pt)
            return pts

        def emit_pv(hp, w, pts, vt, smb, pob):
            ptA, ptB = pts
            pvt = p2ps.tile([128, 196], F32, tag="pvps", name="pvps", bufs=1)
            smt = p2ps.tile([128, TT], F32, tag="stps", name="smps", bufs=3)
            for u in range(2):
                vbase = (w * 2 + u) * 512 + (hp % 4) * 128
                nc.tensor.matmul(
                    pvt[0:64, :], vt[:, vbase:vbase + 64],
                    ptA[:, u * 196:(u + 1) * 196],
                    start=(u == 0), stop=(u == 1),
                    tile_position=(0, 0), skip_group_check=True)
                nc.tensor.matmul(
                    pvt[64:128, :], vt[:, vbase + 64:vbase + 128],
                    ptB[:, u * 196:(u + 1) * 196],
                    start=(u == 0), stop=(u == 1),
                    tile_position=(0, 64), skip_group_check=True)
            for u in range(2):
                nc.tensor.matmul(
                    smt[:, 0:196], ones_bf[0:98, :],
                    ptA[:, u * 196:(u + 1) * 196],
                    start=(u == 0), stop=(u == 1), skip_group_check=True)
            for u in range(2):
                nc.tensor.matmul(
                    smt[:, 196:392], ones_bf[0:98, :],
                    ptB[:, u * 196:(u + 1) * 196],
                    start=(u == 0), stop=(u == 1), skip_group_check=True)
            nc.vector.tensor_copy(out=smb[:, w * TT:(w + 1) * TT], in_=smt)
            nc.vector.tensor_copy(out=pob[:, w * 196:(w + 1) * 196], in_=pvt)

        def emit_den_muls(hp, smb, pob):
            # batched softmax denominators for all 8 windows of this hp:
            # 1/s = exp(-ln(s)); rs2b packs head A on parts 0:64, B on 64:128
            nc.scalar.activation(out=smb, in_=smb, func=AF.Ln)
            lt4 = smb.rearrange("p (w h q) -> p w h q", w=N_WIN, h=2)
            rs2b = p2.tile([128, 8 * 196], F32, tag="rs2b", name="rs2b",
                           bufs=1)
            rs4 = rs2b.rearrange("p (w q) -> p w q", w=N_WIN)
            nc.scalar.activation(out=rs4[0:64, :, :], in_=lt4[0:64, :, 0, :],
                                 func=AF.Exp, scale=-1.0)
            nc.scalar.activation(out=rs4[64:128, :, :],
                                 in_=lt4[64:128, :, 1, :],
                                 func=AF.Exp, scale=-1.0)
            for w in range(N_WIN):
                nc.vector.tensor_mul(
                    out=win_view(xattn[hp], w),
                    in0=pob[:, w * 196:(w + 1) * 196].rearrange(
                        "p (r c) -> p r c", r=WS),
                    in1=rs2b[:, w * 196:(w + 1) * 196].rearrange(
                        "p (r c) -> p r c", r=WS))

        def emit_windows(hp, rel_hp, vchains):
            qrA, qrB = qrelA[hp % 2], qrelB[hp % 2]
            kA, kB = kindA[hp % 2], kindB[hp % 2]
            vt = v_tiles[hp // 4]
            smb = p2.tile([128, 8 * TT], F32, tag="smb", name="smb", bufs=1)
            pob = p2.tile([128, 8 * 196], F32, tag="pob", name="pob", bufs=2)
            rel_steps = {1: 0, 3: 1, 5: 2, 7: 3} if rel_hp is not None else {}
            vchains = list(vchains)
            pts_q = {}
            for w in range(N_WIN + 2):
                if w < N_WIN:
                    pts_q[w] = emit_scores(hp, w, qrA, qrB, kA, kB)
                if w >= 2:
                    emit_pv(hp, w - 2, pts_q.pop(w - 2), vt, smb, pob)
                if w in rel_steps:
                    emit_rel_group(rel_steps[w],
                                   qrelA[rel_hp % 2], qrelB[rel_hp % 2])
                for _ in range(min(2, len(vchains))):
                    vs, vw, vu = vchains.pop(0)
                    emit_v_chain(vs, v_tiles[vs], vw, vu)
            emit_den_muls(hp, smb, pob)

        v_tiles = {}
        v_tiles[0] = v_alloc(0)
        for grp in range(4):
            emit_rel_group(grp, qrelA[0], qrelB[0])
        v_sched = {0: [(0, w, u) for w in range(N_WIN) for u in (0, 1)],
                   1: [(1, w, u) for w in (0, 1, 2) for u in (0, 1)],
                   2: [(1, w, u) for w in (3, 4, 5) for u in (0, 1)],
                   3: [(1, w, u) for w in (6, 7) for u in (0, 1)]}
        for hp in range(8):
            if hp + 1 < 8:
                emit_qk(hp + 1)
                emit_qk(9 + hp)
            if hp == 1:
                v_tiles[1] = v_alloc(1)
            emit_windows(hp, rel_hp=hp + 1 if hp + 1 < 8 else None,
                         vchains=v_sched.get(hp, []))
        dump("d_xattn", xattn)
        close(p2ps, pqkps, p2, pvv, pqk, pwq)

        # ---------------- Phase 3: proj + residual ----------------
        px2 = pool("px2", side="right")                     # x2, lives to the end
        p3 = pool("p3", bufs=2)
        p3ps = pspool("p3ps", bufs=2)
        x2 = []
        for ot in range(NC_C):
            x2.append(px2.tile([128, T], BF16,
                               tag=f"x2_{ot}", name=f"x2_{ot}"))
        wp = []
        for ci in range(NC_C):
            t = p3.tile([128, C], BF16, tag=f"wproj{ci}", name=f"wproj{ci}",
                        bufs=1)
            nc.sync.dma_start(out=t, in_=projwT[ci * 128:(ci + 1) * 128, :])
            wp.append(t)
        for tt in range(NTT):
            for ot in range(NC_C):
                pst = p3ps.tile([128, TT], F32, tag="projps", name="projps", bufs=3)
                for ci in range(NC_C):
                    nc.tensor.matmul(
                        pst, wp[ci][:, ot * 128:(ot + 1) * 128],
                        xattn[ci][:, tt * TT:(tt + 1) * TT],
                        start=(ci == 0), stop=(ci == NC_C - 1))
                tmp = p3.tile([128, TT], F32, tag="projtmp", name="projtmp")
                nc.scalar.activation(out=tmp, in_=pst, func=AF.Identity,
                                     bias=projb_sb[:, ot:ot + 1])
                xre = p3.tile([128, TT], BF16, tag="xre", name="xre")
                nc.sync.dma_start(
                    out=xre,
                    in_=xT[ot * 128:(ot + 1) * 128, tt * TT:(tt + 1) * TT])
                nc.vector.tensor_add(
                    out=x2[ot][:, tt * TT:(tt + 1) * TT], in0=tmp, in1=xre)
        dump("d_x2", x2)
        close(p3ps, p3, pc, pa)

        # ---------------- Phase 4: LN2 + MLP ----------------
        p4x = pool("p4x")                     # xln2
        p4w = pool("p4w", bufs=2)             # fc weight rings (DMA early)
        p4a = pool("p4a", bufs=2)
        p4aps = pspool("p4aps", bufs=2)

        def load_w1(og):
            w1 = []
            for ci in range(NC_C):
                t = p4w.tile([128, 512], BF16, tag=f"w1_{ci}",
                             name=f"w1_{ci}", bufs=2)
                nc.sync.dma_start(
                    out=t, in_=fc1wT[ci * 128:(ci + 1) * 128,
                                     og * 512:(og + 1) * 512])
                w1.append(t)
            return w1

        w1_pre = {0: load_w1(0), 1: load_w1(1)}
        xln2 = []
        for ci in range(NC_C):
            xln2.append(p4x.tile([128, T], BF16, tag=f"xln2_{ci}",
                                 name=f"xln2_{ci}"))

        def apply_ln2(sl, pl):
            for ci in range(NC_C):
                tmp = pl.tile([128, 392], F32, tag="lntmp", name="lntmp",
                              bufs=3)
                nc.vector.tensor_sub(out=tmp, in0=x2[ci][:, sl],
                                     in1=mb[:, sl])
                nc.vector.tensor_mul(out=xln2[ci][:, sl], in0=tmp,
                                     in1=rsb[:, sl])

        # stats for all chunks, but apply only chunks 0/1 now: fc1's tp=0
        # pass needs just those, so chunks 2/3 apply under the fc1 og loop
        for tt in range(NTT):
            sl = slice(tt * TT, (tt + 1) * TT)
            ln_chunk(nc, p4a, p4aps, x2, sl, mb, rsb, 1.0 / C, eps5)
            if tt < 2:
                apply_ln2(sl, p4a)
        dump("d_xln2", xln2)
        close(p4aps, p4a)

        p4 = pool("p4", bufs=2)
        p4ps = pspool("p4ps", bufs=2)
        hbuf = p4.tile([128, 32 * 784], BF16, tag="hbuf", name="hbuf", bufs=1)
        for tp in range(2):
            for og in range(8):
                w1 = (w1_pre[og] if (tp == 0 and og in w1_pre)
                      else load_w1(og))
                for tl in range(2):
                    for otl in range(4):
                        ot = og * 4 + otl
                        t0 = tp * 784 + tl * TT
                        pst = p4ps.tile([128, TT], F32, tag="fc1ps",
                                        name="fc1ps", bufs=3)
                        for ci in range(NC_C):
                            nc.tensor.matmul(
                                pst, w1[ci][:, otl * 128:(otl + 1) * 128],
                                xln2[ci][:, t0:t0 + TT],
                                start=(ci == 0), stop=(ci == NC_C - 1))
                        nc.scalar.activation(
                            out=hbuf[:, ot * 784 + tl * TT:
                                     ot * 784 + (tl + 1) * TT],
                            in_=pst, func=AF.Gelu,
                            bias=fc1b_sb[:, ot:ot + 1])
                if tp == 0 and og < 2:
                    apply_ln2(slice((2 + og) * TT, (3 + og) * TT), p4w)
            for og2 in range(4):
                w2 = []
                for ki in range(32):
                    t = p4w.tile([128, 256], BF16, tag=f"w2_{ki}",
                                 name=f"w2_{ki}", bufs=2)
                    nc.sync.dma_start(
                        out=t, in_=fc2wT[ki * 128:(ki + 1) * 128,
                                         og2 * 256:(og2 + 1) * 256])
                    w2.append(t)
                for otl in range(2):
                    ot = og2 * 2 + otl
                    for tl in range(2):
                        t0 = tp * 784 + tl * TT
                        pst = p4ps.tile([128, TT], F32, tag="fc2ps",
                                        name="fc2ps", bufs=3)
                        for ki in range(32):
                            nc.tensor.matmul(
                                pst, w2[ki][:, otl * 128:(otl + 1) * 128],
                                hbuf[:, ki * 784 + tl * TT:
                                     ki * 784 + (tl + 1) * TT],
                                start=(ki == 0), stop=(ki == 31))
                        tmp = p4.tile([128, TT], F32, tag="fc2tmp",
                                      name="fc2tmp")
                        nc.scalar.activation(
                            out=tmp, in_=pst, func=AF.Identity,
                            bias=fc2b_sb[:, ot:ot + 1])
                        nc.vector.tensor_add(
                            out=x2[ot][:, t0:t0 + TT],
                            in0=tmp, in1=x2[ot][:, t0:t0 + TT])
        if debug:
            for ki in range(32):
                nc.sync.dma_start(out=dbg["d_h"][ki * 128:(ki + 1) * 128, :],
                                  in_=hbuf[:, ki * 784:(ki + 1) * 784])
        dump("d_x3", x2)
        close(p4ps, p4, p4w, p4x)

        # ---------------- Phase 5: ResBottleneckBlock ----------------
        pcw = pool("pcw")                     # all conv weights, DMA'd early
        wc1 = []
        for ci in range(NC_C):
            t = pcw.tile([128, 512], BF16, tag=f"wc1_{ci}", name=f"wc1_{ci}")
            nc.sync.dma_start(out=t, in_=c1wT[ci * 128:(ci + 1) * 128, :])
            wc1.append(t)
        wc2 = {}
        for tap in range(9):
            for ci in range(4):
                t = pcw.tile([128, 512], BF16, tag=f"wc2_{tap}_{ci}",
                             name=f"wc2_{tap}_{ci}")
                nc.sync.dma_start(
                    out=t, in_=c2wT[tap, ci * 128:(ci + 1) * 128, :])
                wc2[(tap, ci)] = t
        wc3 = []
        for ci in range(4):
            t = pcw.tile([128, C], BF16, tag=f"wc3_{ci}", name=f"wc3_{ci}")
            nc.sync.dma_start(out=t, in_=c3wT[ci * 128:(ci + 1) * 128, :])
            wc3.append(t)
        pr1 = pool("pr1")                     # r1pad, lives 5a..5b
        r1pad = []
        for ci in range(4):
            t = pr1.tile([128, BS * 900 + 2], BF16, tag=f"r1pad{ci}",
                         name=f"r1pad{ci}")
            nc.gpsimd.memset(t, 0.0)
            r1pad.append(t)
        p5a = pool("p5a", bufs=2)
        p5aps = pspool("p5aps", bufs=2)
        c1s = [p5a.tile([128, T], BF16, tag=f"c1s{ot}",
                        name=f"c1s{ot}", bufs=1) for ot in range(4)]
        def apply_n1(sl):
            tt = sl.start // TT
            i, wh = tt // 2, tt % 2
            for ci in range(4):
                pv = r1pad[ci][:, 0:1800].rearrange(
                    "p (i y x) -> p i y x", i=BS, y=30)
                tmp = p5a.tile([128, TT], F32, tag="c1tmp", name="c1tmp",
                               bufs=3)
                nc.vector.tensor_sub(out=tmp, in0=c1s[ci][:, sl],
                                     in1=mb[:, sl])
                nc.vector.tensor_mul(out=tmp, in0=tmp, in1=rsb[:, sl])
                nc.scalar.activation(
                    out=pv[:, i, 1 + 14 * wh:15 + 14 * wh, 1:29],
                    in_=tmp, func=AF.Gelu,
                    bias=n1b_sb[:, ci:ci + 1], scale=n1w_sb[:, ci:ci + 1])

        for tt in range(NTT):
            sl = slice(tt * TT, (tt + 1) * TT)
            for ot in range(4):
                pst = p5aps.tile([128, TT], F32, tag="c1ps", name="c1ps",
                                 bufs=3)
                for ci in range(NC_C):
                    nc.tensor.matmul(
                        pst, wc1[ci][:, ot * 128:(ot + 1) * 128],
                        x2[ci][:, sl],
                        start=(ci == 0), stop=(ci == NC_C - 1))
                nc.vector.tensor_copy(out=c1s[ot][:, sl], in_=pst)
            ln_chunk(nc, p5a, p5aps, c1s, sl, mb, rsb, 1.0 / 512, eps6)
            apply_n1(sl)
        dump("d_c1", c1s)
        dump("d_r1p", r1pad)
        close(p5aps, p5a)

        pr2 = pool("pr2", side="right")                     # r2, lives 5b..5c
        p5b = pool("p5b", bufs=2)
        p5bps = pspool("p5bps", bufs=2)
        c2s = [p5b.tile([128, T], BF16, tag=f"c2s{ot}",
                        name=f"c2s{ot}", bufs=1) for ot in range(4)]
        def conv2_chunk(tt):
            for ot in range(4):
                i, wh = tt // 2, tt % 2
                pst = p5bps.tile([128, 420], F32, tag="c2ps", name="c2ps", bufs=3)
                n9 = 0
                for tap in range(9):
                    dy, dx = tap // 3, tap % 3
                    for ci in range(4):
                        st0 = i * 900 + (14 * wh + dy) * 30 + dx
                        nc.tensor.matmul(
                            pst, wc2[(tap, ci)][:, ot * 128:(ot + 1) * 128],
                            r1pad[ci][:, st0:st0 + 420],
                            start=(n9 == 0), stop=(n9 == 35))
                        n9 += 1
                nc.scalar.copy(
                    out=c2s[ot][:, tt * TT:(tt + 1) * TT].rearrange(
                        "p (h x) -> p h x", h=WS),
                    in_=pst.rearrange("p (h x) -> p h x", h=WS)[:, :, 0:28])
        r2 = []
        for ci in range(4):
            r2.append(pr2.tile([128, T], BF16, tag=f"r2_{ci}",
                               name=f"r2_{ci}"))

        def apply_n2(sl):
            for ci in range(4):
                tmp = p5b.tile([128, TT], F32, tag="c2tmp", name="c2tmp",
                               bufs=3)
                nc.vector.tensor_sub(out=tmp, in0=c2s[ci][:, sl],
                                     in1=mb[:, sl])
                nc.vector.tensor_mul(out=tmp, in0=tmp, in1=rsb[:, sl])
                nc.scalar.activation(
                    out=r2[ci][:, sl], in_=tmp, func=AF.Gelu,
                    bias=n2b_sb[:, ci:ci + 1], scale=n2w_sb[:, ci:ci + 1])

        for tt in range(NTT):
            sl = slice(tt * TT, (tt + 1) * TT)
            conv2_chunk(tt)
            ln_chunk(nc, p5b, p5bps, c2s, sl, mb, rsb, 1.0 / 512, eps6)
            apply_n2(sl)
        dump("d_c2", c2s)
        close(p5bps, p5b, pr1)

        p5c = pool("p5c", bufs=2)
        p5cps = pspool("p5cps", bufs=2)
        c3s = [p5c.tile([128, T], BF16, tag=f"c3s{ot}",
                        name=f"c3s{ot}", bufs=1) for ot in range(NC_C)]
        def apply_n3(s):
            # spread across DVE (sub/mul/scale), gpsimd (residual)
            for ot in range(NC_C):
                tmp = p5c.tile([128, s.stop - s.start], F32, tag="ytmp",
                               name="ytmp", bufs=3)
                nc.vector.tensor_sub(out=tmp, in0=c3s[ot][:, s], in1=mb[:, s])
                nc.vector.tensor_mul(out=tmp, in0=tmp, in1=rsb[:, s])
                if zero_bias:
                    nc.vector.tensor_scalar_mul(out=tmp, in0=tmp,
                                                scalar1=n3w_sb[:, ot:ot + 1])
                else:
                    nc.scalar.activation(
                        out=tmp, in_=tmp, func=AF.Identity,
                        scale=n3w_sb[:, ot:ot + 1], bias=n3b_sb[:, ot:ot + 1])
                yt = p5c.tile([128, s.stop - s.start], F32, tag="yt",
                              name="yt", bufs=3)
                nc.gpsimd.tensor_add(out=yt, in0=tmp, in1=x2[ot][:, s])
                nc.sync.dma_start(out=yT[ot * 128:(ot + 1) * 128, s], in_=yt)

        for tt in range(NTT):
            sl = slice(tt * TT, (tt + 1) * TT)
            for ot in range(NC_C):
                pst = p5cps.tile([128, TT], F32, tag="c3ps", name="c3ps",
                                 bufs=3)
                for ci in range(4):
                    nc.tensor.matmul(
                        pst, wc3[ci][:, ot * 128:(ot + 1) * 128],
                        r2[ci][:, sl],
                        start=(ci == 0), stop=(ci == 3))
                nc.scalar.copy(out=c3s[ot][:, sl], in_=pst)
            ln_chunk(nc, p5c, p5cps, c3s, sl, mb, rsb, 1.0 / C, eps6)
            apply_n3(sl)
        dump("d_c3", c3s)
        close(p5cps, p5c, pcw, pr2, px2, g)

    return nc


# ---------------------------------------------------------------------------
# Host side
# ---------------------------------------------------------------------------
_program_cache = {}


def _get_program(zero_bias=False):
    key = ("nc", zero_bias)
    if key not in _program_cache:
        _program_cache[key] = build_program(zero_bias=zero_bias)
    return _program_cache[key]


def _bf(x):
    return np.ascontiguousarray(x).astype(ml_dtypes.bfloat16)


def prep_inputs(inputs):
    """Build the per-core input maps (host-side sharding + weight prep)."""
    f = {k: np.asarray(v, dtype=np.float32) for k, v in inputs.items()}
    scale = HD ** -0.5

    qkv_w = f["qkv_w"].copy()          # [3C, C]
    qkv_b = f["qkv_b"].copy()          # [3C]
    qkv_w[:C] *= scale                 # fold 1/sqrt(hd) into q
    qkv_b[:C] *= scale
    # fold ln1 affine into qkv
    qkv_wT = (qkv_w * f["ln1_w"][None, :]).T.copy()      # [C, 3C]
    qkv_b_eff = qkv_b + qkv_w @ f["ln1_b"]
    # fold ln2 affine into fc1
    fc1_wT = (f["fc1_w"] * f["ln2_w"][None, :]).T.copy()  # [C, MLP]
    fc1_b_eff = f["fc1_b"] + f["fc1_w"] @ f["ln2_b"]

    proj_wT = f["proj_w"].T.copy()
    fc2_wT = f["fc2_w"].T.copy()
    c1_wT = f["conv1_w"][:, :, 0, 0].T.copy()            # [C, C/2]
    c2 = f["conv2_w"]                                    # [O, I, 3, 3]
    c2_wT = np.ascontiguousarray(
        c2.transpose(2, 3, 1, 0).reshape(9, C // 2, C // 2))
    c3_wT = f["conv3_w"][:, :, 0, 0].T.copy()            # [C/2, C]

    # rel pos tables: rhe[p, qh*14+kh] = 8*rel_pos_h[qh-kh+13, p%64]
    rh8 = 8.0 * f["rel_pos_h"]                           # [27, 64]
    rw8 = 8.0 * f["rel_pos_w"]
    qh_i, kh_i = np.meshgrid(np.arange(WS), np.arange(WS), indexing="ij")
    idx = qh_i - kh_i + WS - 1                           # [qh, kh]
    rhe = rh8[idx]                                       # [qh, kh, 64]
    rwe = rw8[idx]
    rhe_t = np.zeros((128, 196), np.float32)
    rwe_t = np.zeros((128, 196), np.float32)
    rhe_flat = rhe.transpose(2, 0, 1).reshape(64, 196)   # [c, qh*14+kh]
    rwe_flat = rwe.transpose(2, 0, 1).reshape(64, 196)
    rhe_t[0:64] = rhe_flat
    rhe_t[64:128] = rhe_flat
    rwe_t[0:64] = rwe_flat
    rwe_t[64:128] = rwe_flat

    # indicators vs rel rows: A-kh 0:14, B-kh 32:46, A-kw 64:78, B-kw 96:110
    indA = np.zeros((128, 196), np.float32)
    indB = np.zeros((128, 196), np.float32)
    kt = np.arange(196)
    for j in range(WS):
        indA[j, kt // 14 == j] = 1.0
        indA[64 + j, kt % 14 == j] = 1.0
        indB[32 + j, kt // 14 == j] = 1.0
        indB[96 + j, kt % 14 == j] = 1.0
    # device-side kind/qrel init templates (rows 64:110 of kindA, 0:64 of
    # kindB, plus an 18-row zero slab for the qrel pad rows)
    kindA_init = np.zeros((46, T), np.float32)
    kindB_init = np.zeros((64, T), np.float32)
    for w in range(8):
        cs = slice(w * 196, (w + 1) * 196)
        kindA_init[0:14, cs] = indA[0:14]
        kindA_init[32:46, cs] = indA[64:78]
        kindB_init[0:14, cs] = indB[32:46]
        kindB_init[32:46, cs] = indB[96:110]

    common = {
        "qkvwT": _bf(qkv_wT),
        "qkvb": qkv_b_eff.astype(np.float32),
        "qkvbv": _bf(qkv_b_eff[2 * C:][None, :]),
        "projwT": _bf(proj_wT),
        "projb": f["proj_b"],
        "fc1wT": _bf(fc1_wT),
        "fc1b": fc1_b_eff.astype(np.float32),
        "fc2wT": _bf(fc2_wT),
        "fc2b": f["fc2_b"],
        "c1wT": _bf(c1_wT),
        "c2wT": _bf(c2_wT),
        "c3wT": _bf(c3_wT),
        "n1w": f["n1_w"], "n1b": f["n1_b"],
        "n2w": f["n2_w"], "n2b": f["n2_b"],
        "n3w": f["n3_w"], "n3b": f["n3_b"],
        "rhe": _bf(rhe_t), "rwe": _bf(rwe_t),
        "kindAi": _bf(kindA_init), "kindBi": _bf(kindB_init),
        "zpad": np.zeros((18, T), ml_dtypes.bfloat16),
    }
    x = f["x"]                                           # [B, 28, 28, C]
    in_maps = []
    for core in range(NCORES):
        xs = x[core * BS:(core + 1) * BS].reshape(T, C).T  # [C, T]
        m = dict(common)
        m["xT"] = _bf(xs)
        in_maps.append(m)
    return in_maps


def run(inputs, trace=False):
    from concourse.bass_utils import run_bass_kernel_spmd

    in_maps = prep_inputs(inputs)
    zb = bool(np.all(in_maps[0]["qkvb"] == 0.0)
              and np.all(in_maps[0]["n3b"] == 0.0))
    nc = _get_program(zero_bias=zb)
    res = run_bass_kernel_spmd(nc, in_maps, core_ids=list(range(NCORES)),
                               trace=trace)
    outs = []
    for core in range(NCORES):
        yt = res.results[core]["yT"]                     # [C, T]
        outs.append(yt.T.reshape(BS, HH, WW, C))
    y = np.concatenate(outs, axis=0).astype(np.float32)
    return y, res


def kernel(**inputs):
    y, _ = run(inputs, trace=False)
    return y



# revision 92
# speedup vs baseline: 1.0156x; 1.0052x over previous
"""Trainium2 Bass kernel for nn_Extractor_Processor_75368086110414.

Windowed-attention transformer block (ViTDet-style) + ResBottleneckBlock,
data-parallel over batch across 8 NeuronCores (2 images per core).

Device layout: activations live TRANSPOSED in SBUF as [feature, token] with
features on partitions, tokens (row-major per image) on the free dim. All
matmuls put the contraction dim on partitions (W^T tiles pre-transposed on
host). LayerNorm statistics (over channels = partitions) are computed with
ones-matmuls on the tensor engine; per-token stats are broadcast back across
partitions via a DMA round-trip through DRAM scratch.

Attention avoids materializing any transposes: S^T = k q^T (+ decomposed
rel-pos bias injected through an indicator matmul accumulated in PSUM),
exp on ACT, out^T = (P^T)^T-free col-tiled PV matmuls, and softmax
denominators from ones-matmul column sums of P^T.
"""

import json
import sys
import types

import numpy as np
import ml_dtypes

import concourse.bass as bass
import concourse.tile as tile
from concourse import mybir
from concourse.vector_clock import ScopedClock
from concourse.masks import make_identity

F32 = mybir.dt.float32
BF16 = mybir.dt.bfloat16
AF = mybir.ActivationFunctionType
OP = mybir.AluOpType

# ---------------------------------------------------------------------------
# Patch 1: the pinned walrus rejects >1 sync wait per instruction. Split the
# kernel-tail drain's waits across a chain of drains, and post-process the
# BIR JSON to peel extra waits off any instruction onto injected NoOps.
# ---------------------------------------------------------------------------
MAX_WAITS = 1
_patched = False


def _drain_and_barrier(self, tick_clock, wait_clock):
    nc = self.nc
    drain_inst = nc.sync.drain()
    wait_clock.add_sem_waits(
        drain_inst.ins, ScopedClock({None: tick_clock.global_clock})
    )
    waits = list(drain_inst.ins.sync_info.on_wait)
    if len(waits) > MAX_WAITS:
        drain_inst.ins.sync_info = mybir.SyncInfo(
            on_wait=waits[:MAX_WAITS], on_update=[]
        )
        rest = waits[MAX_WAITS:]
        for i in range(0, len(rest), MAX_WAITS):
            extra = nc.sync.drain()
            extra.ins.sync_info = mybir.SyncInfo(
                on_wait=rest[i : i + MAX_WAITS], on_update=[]
            )
    nc.all_engine_barrier()
    assert self.sems is not None
    popped = nc._tile_sem_poison_stack.pop()
    assert popped is self._sem_poison
    nc.clear_and_free_semaphores(list(self.sems.allocated().values()))
    nc.all_engine_barrier()


def _split_waits_json(data: bytes) -> bytes:
    bj = json.loads(data)
    counter = [0]
    changed = False
    for fn in bj.get("functions", []):
        for bb in fn.get("blocks", []):
            insts = bb.get("instructions")
            if not insts:
                continue
            out = []
            for inst in insts:
                si = inst.get("sync_info")
                waits = si.get("on_wait") if si else None
                if waits and len(waits) > MAX_WAITS:
                    keep = waits[-MAX_WAITS:]
                    rest = waits[:-MAX_WAITS]
                    for i in range(0, len(rest), MAX_WAITS):
                        counter[0] += 1
                        out.append({
                            "debug": inst.get("debug"),
                            "engine": inst["engine"],
                            "ins": [],
                            "name": f"I-ws{counter[0]}",
                            "opcode": "NoOp",
                            "outs": [],
                            "sync_info": {
                                "on_wait": rest[i : i + MAX_WAITS],
                                "on_update": [],
                            },
                        })
                    si["on_wait"] = keep
                    changed = True
                out.append(inst)
            bb["instructions"] = out
    if not changed:
        return data
    return json.dumps(bj).encode()


def _apply_patches():
    global _patched
    if _patched:
        return
    _patched = True
    tile.TileContext._drain_and_barrier = _drain_and_barrier
    orig = bass.Bass.to_json_bytes
    bass.Bass.to_json_bytes = lambda self, *a, **kw: _split_waits_json(
        orig(self, *a, **kw)
    )
    # Patch 2: the agent image's antenv lacks axon_hooks; register a shim so
    # run_bass_kernel_spmd(trace=True) can find the NTFF profile hook.
    if "antenv.axon_hooks" not in sys.modules:
        try:
            from trn_agent_boot.trn_boot import _ntff_profile_via_ctypes

            hook = _ntff_profile_via_ctypes("/opt/axon/libaxon_pjrt.so")
        except Exception:
            hook = None
        mod = types.ModuleType("antenv.axon_hooks")
        mod.get_axon_ntff_profile_hook = lambda: hook
        mod.set_axon_ntff_profile_hook = lambda h: None
        sys.modules["antenv.axon_hooks"] = mod


_apply_patches()

# ---------------------------------------------------------------------------
# Problem geometry (hardcoded per spec)
# ---------------------------------------------------------------------------
C = 1024
NH = 16
HD = 64
WS = 14
MLP = 4096
B, HH, WW = 16, 28, 28
NCORES = 8
BS = B // NCORES          # images per core
T = BS * HH * WW          # 1568 tokens per core
TT = 392                  # token tile (one 14-row window-band of one image)
NTT = T // TT             # 4
NC_C = C // 128           # 8 c-tiles
N_WIN = BS * 4            # 8 windows per core
N_CHUNK = 16              # LN stat chunks of 98 tokens (window halves)


def _win_base(w):
    i, wh, ww = w // 4, (w // 2) % 2, w % 2
    return i * 784 + wh * 392 + ww * 14, (i, wh, ww)


def _r4(t):
    # [128, T] -> [p, i, wh, r, w28]
    return t.rearrange("p (i wh r w) -> p i wh r w", i=BS, wh=2, r=WS)


def win_view(t, w):
    i, wh, ww = w // 4, (w // 2) % 2, w % 2
    return _r4(t)[:, i, wh, :, ww * 14:(ww + 1) * 14]          # [p,14,14]


def half_view(t, w, u):
    i, wh, ww = w // 4, (w // 2) % 2, w % 2
    return _r4(t)[:, i, wh, u * 7:(u + 1) * 7, ww * 14:(ww + 1) * 14]  # [p,7,14]


def chunk_view(t, u):
    # flat contiguous 98-token chunk u (any token partition works for
    # per-token stats over channels)
    return t[:, u * 98:(u + 1) * 98]


def pair_view(t, i, wh, hf):
    # two chunks (ww=0,1) as [p, rr7, ww2, c14] but ordered (ww, rr, c) to
    # match a flat (ww-major) source stream
    r6 = t.rearrange("p (i wh hf rr ww c) -> p i wh hf rr ww c",
                     i=BS, wh=2, hf=2, rr=7, ww=2)
    return r6[:, i, wh, hf, :, :, :].transpose([0, 2, 1, 3])   # [p, ww2, rr7, c14]


# ---------------------------------------------------------------------------
# LayerNorm-over-partitions helper
# ---------------------------------------------------------------------------
def ln_chunk(nc, sbp, psp, src_tiles, sl, mb, rsb, inv_c, eps_tile):
    """Emit squares + ones-matmul stats + mean/rsqrt epilogue for one
    392-column chunk. Squares go to bf16 ring tiles so the sq-sum matmuls
    run at bf16 rate; the sum matmuls consume the source tiles directly."""
    ncti = len(src_tiles)
    w = sl.stop - sl.start
    F32R = mybir.dt.float32r
    ones_r = globals()["_ones_fr"].bitcast(F32R)
    ones_b = globals()["_ones_bf"]
    src_bf = src_tiles[0].dtype == BF16
    sqs = []
    for ci in range(ncti):
        sq = sbp.tile([128, w], BF16, tag=f"lnsq{ci}", name=f"lnsq{ci}",
                      bufs=2)
        nc.scalar.activation(out=sq, in_=src_tiles[ci][:, sl],
                             func=AF.Square)
        sqs.append(sq)
    ps_s = psp.tile([128, w], F32, tag="lnsum", name="lnsum", bufs=2)
    ps_q = psp.tile([128, w], F32, tag="lnsqp", name="lnsqp", bufs=2)
    for ci in range(ncti):
        nc.tensor.matmul(ps_s, ones_b if src_bf else ones_r,
                         src_tiles[ci][:, sl],
                         start=(ci == 0), stop=(ci == ncti - 1))
    for ci in range(ncti):
        nc.tensor.matmul(ps_q, ones_b, sqs[ci],
                         start=(ci == 0), stop=(ci == ncti - 1))
    nc.scalar.activation(out=mb[:, sl], in_=ps_s, func=AF.Copy,
                         scale=inv_c)
    msq = sbp.tile([128, w], F32, tag="lnmsq", name="lnmsq", bufs=2)
    nc.vector.tensor_mul(out=msq, in0=mb[:, sl], in1=mb[:, sl])
    sqm = sbp.tile([128, w], F32, tag="lnsqm", name="lnsqm", bufs=2)
    nc.scalar.activation(out=sqm, in_=ps_q, func=AF.Copy, scale=inv_c)
    ve = sbp.tile([128, w], F32, tag="lnve", name="lnve", bufs=2)
    nc.vector.tensor_sub(out=ve, in0=sqm, in1=msq)
    nc.scalar.activation(out=ve, in_=ve, func=AF.Ln, bias=eps_tile)
    nc.scalar.activation(out=rsb[:, sl], in_=ve, func=AF.Exp,
                         scale=-0.5)


def ln_stats(nc, pools, src_tiles, scratch, mb, rsb, inv_c, eps_tile, ident,
             apply_cb=None):
    sbp, psp = pools
    for t0 in range(0, T, 392):
        sl = slice(t0, t0 + 392)
        ln_chunk(nc, sbp, psp, src_tiles, sl, mb, rsb, inv_c, eps_tile)
        if apply_cb is not None:
            apply_cb(sl)


# ---------------------------------------------------------------------------
# Program builder
# ---------------------------------------------------------------------------
ones_f32 = None  # set inside build


def build_program(debug=False, zero_bias=False):
    global ones_f32
    nc = bass.Bass()

    def din(name, shape, dt=F32):
        return nc.declare_dram_parameter(name, shape, dt, isOutput=False)

    xT = din("xT", [C, T], BF16)
    qkvwT = din("qkvwT", [C, 3 * C], BF16)
    qkvb = din("qkvb", [3 * C])
    qkvbv = din("qkvbv", [1, C], BF16)
    projwT = din("projwT", [C, C], BF16)
    projb = din("projb", [C])
    fc1wT = din("fc1wT", [C, MLP], BF16)
    fc1b = din("fc1b", [MLP])
    fc2wT = din("fc2wT", [MLP, C], BF16)
    fc2b = din("fc2b", [C])
    c1wT = din("c1wT", [C, C // 2], BF16)
    c2wT = din("c2wT", [9, C // 2, C // 2], BF16)
    c3wT = din("c3wT", [C // 2, C], BF16)
    n1w = din("n1w", [C // 2])
    n1b = din("n1b", [C // 2])
    n2w = din("n2w", [C // 2])
    n2b = din("n2b", [C // 2])
    n3w = din("n3w", [C])
    n3b = din("n3b", [C])
    rhe = din("rhe", [128, 196], BF16)
    rwe = din("rwe", [128, 196], BF16)
    kindAi = din("kindAi", [46, T], BF16)
    kindBi = din("kindBi", [64, T], BF16)
    zpad = din("zpad", [18, T], BF16)
    yT = nc.declare_dram_parameter("yT", [C, T], F32, isOutput=True)
    dbg = {}
    if debug:
        for name, shape, dt in [
            ("d_xln1", [C, T], BF16), ("d_qk", [2 * C, T], BF16),
            ("d_v", [98, N_WIN * 2 * C], BF16), ("d_rel", [128, T], BF16),
            ("d_xattn", [C, T], BF16), ("d_x2", [C, T], F32),
            ("d_xln2", [C, T], BF16), ("d_h", [MLP, 784], BF16),
            ("d_x3", [C, T], F32),
            ("d_c1", [C // 2, T], F32), ("d_r1p", [C // 2, BS * 900 + 2], BF16),
            ("d_c2", [C // 2, T], F32), ("d_c3", [C, T], F32),
            ("d_mb", [128, T], F32), ("d_rsb", [128, T], F32),
        ]:
            dbg[name] = nc.declare_dram_parameter(name, shape, dt, isOutput=True)

    def dump(name, tiles, rows=128):
        if not debug:
            return
        d = dbg[name]
        for i, t in enumerate(tiles):
            nc.sync.dma_start(out=d[i * rows:(i + 1) * rows, :][0:rows, :],
                              in_=t[0:rows, :] if rows != 98 else t)


    scratch = nc.dram_tensor("scratch", [2 * 98 * N_CHUNK], F32)
    scratch2 = nc.dram_tensor("scratch2", [64 * 392], F32)

    _cms = {}

    def pool(name, bufs=1, side=None):
        cm = tc.tile_pool(name=name, bufs=bufs, side=side)
        p = cm.__enter__()
        _cms[id(p)] = cm
        return p

    def pspool(name, bufs=1):
        cm = tc.tile_pool(name=name, bufs=bufs, space="PSUM")
        p = cm.__enter__()
        _cms[id(p)] = cm
        return p

    def close(*pools_):
        for p in pools_:
            _cms.pop(id(p)).__exit__(None, None, None)

    with tile.TileContext(nc) as tc:
        g = pool("glob")
        global ones_f32
        ones_f32 = g.tile([128, 1], F32, tag="ones_f32", name="ones_f32")
        nc.vector.memset(ones_f32, 1.0)
        ones_bf = g.tile([128, 128], BF16, tag="ones_bf", name="ones_bf")
        nc.vector.memset(ones_bf, 1.0)
        ones_fr = g.tile([128, 128], F32, tag="ones_fr", name="ones_fr")
        nc.vector.memset(ones_fr, 1.0)
        globals()["_ones_fr"] = ones_fr
        globals()["_ones_bf"] = ones_bf
        eps5 = g.tile([128, 1], F32, tag="eps5", name="eps5")
        nc.vector.memset(eps5, 1e-5)
        eps6 = g.tile([128, 1], F32, tag="eps6", name="eps6")
        nc.vector.memset(eps6, 1e-6)
        ident = g.tile([128, 128], F32, tag="ident", name="ident")
        make_identity(nc, ident)

        pa = pool("pa")                       # xln1, lives through qkv
        pc = pool("pc")                       # xattn, lives thru proj
        pwq = pool("pwq")                     # qkv weights (DMA'd early)
        p1x = pool("p1x")                     # raw x, dies after apply
        p1 = pool("p1", bufs=2)
        pqkps = pspool("pqkps", bufs=1)       # qk/v chain ring (2 banks),
        p1ps = pspool("p1ps", bufs=2)         # coexists with LN1 psum
        F32R = mybir.dt.float32r
        xt = []
        for ci in range(NC_C):
            xt.append(p1x.tile([128, T], BF16, tag=f"xt{ci}", name=f"xt{ci}"))
        for half in range(2):
            sl = slice(half * 2 * TT, (half + 1) * 2 * TT)
            for ci in range(NC_C):
                nc.sync.dma_start(out=xt[ci][:, sl],
                                  in_=xT[ci * 128:(ci + 1) * 128, sl])
        # rel-pos tables first (tiny)
        rhe_sb = g.tile([128, 196], BF16, tag="rhe_sb", name="rhe_sb")
        nc.sync.dma_start(out=rhe_sb, in_=rhe[:, :])
        rwe_sb = g.tile([128, 196], BF16, tag="rwe_sb", name="rwe_sb")
        nc.sync.dma_start(out=rwe_sb, in_=rwe[:, :])
        # qkv weights next on the queue: quarters q0 (q heads 0-7) and q2
        # (k heads 0-7) first so qk(0) can start as soon as LN1 drains.
        wq = []
        for ci in range(NC_C):
            wq.append(pwq.tile([128, 2 * C], BF16, tag=f"wqk{ci}",
                               name=f"wqk{ci}"))
        wv = []
        for ci in range(NC_C):
            wv.append(pwq.tile([128, C], BF16, tag=f"wv{ci}", name=f"wv{ci}"))
        for quad in (0, 2):
            for ci in range(NC_C):
                nc.sync.dma_start(
                    out=wq[ci][:, quad * 512:(quad + 1) * 512],
                    in_=qkvwT[ci * 128:(ci + 1) * 128,
                              quad * 512:(quad + 1) * 512])

        def stage_bias(src, n, name):
            t = g.tile([128, n], F32, tag=name, name=name)
            nc.sync.dma_start(out=t, in_=src.rearrange("(o p) -> p o", p=128))
            return t

        qkvb_sb = stage_bias(qkvb, 24, "qkvb_sb")
        for ci in range(NC_C):
            nc.sync.dma_start(
                out=wv[ci], in_=qkvwT[ci * 128:(ci + 1) * 128, 2 * C:3 * C])
        for quad in (1, 3):
            for ci in range(NC_C):
                nc.sync.dma_start(
                    out=wq[ci][:, quad * 512:(quad + 1) * 512],
                    in_=qkvwT[ci * 128:(ci + 1) * 128,
                              quad * 512:(quad + 1) * 512])
        projb_sb = stage_bias(projb, 8, "projb_sb")

        # qrel/kind tiles; pad rows and indicator rows come straight from
        # host-prepared DRAM templates (zero engine time). qrel pad rows
        # must be zeroed: garbage bf16 can be Inf/NaN and 0 (stationary)
        # * Inf = NaN in the accumulation.
        qrelA = [pwq.tile([128, T], BF16, tag=f"qrelA{i}", name=f"qrelA{i}")
                 for i in range(2)]
        qrelB = [pwq.tile([128, T], BF16, tag=f"qrelB{i}", name=f"qrelB{i}")
                 for i in range(2)]
        kindA = [pwq.tile([128, T], BF16, tag=f"kindA{i}", name=f"kindA{i}")
                 for i in range(2)]
        kindB = [pwq.tile([128, T], BF16, tag=f"kindB{i}", name=f"kindB{i}")
                 for i in range(2)]
        for i in range(2):
            nc.sync.dma_start(out=kindA[i][64:110, :], in_=kindAi[:, :])
            nc.sync.dma_start(out=kindB[i][0:64, :], in_=kindBi[:, :])
            nc.sync.dma_start(out=qrelA[i][78:96, :], in_=zpad[:, :])
            nc.sync.dma_start(out=qrelB[i][14:32, :], in_=zpad[:, :])
            nc.sync.dma_start(out=qrelB[i][46:64, :], in_=zpad[:, :])
        fc1b_sb = stage_bias(fc1b, 32, "fc1b_sb")
        fc2b_sb = stage_bias(fc2b, 8, "fc2b_sb")
        n1w_sb = stage_bias(n1w, 4, "n1w_sb")
        n1b_sb = stage_bias(n1b, 4, "n1b_sb")
        n2w_sb = stage_bias(n2w, 4, "n2w_sb")
        n2b_sb = stage_bias(n2b, 4, "n2b_sb")
        n3w_sb = stage_bias(n3w, 8, "n3w_sb")
        n3b_sb = stage_bias(n3b, 8, "n3b_sb")

        mb = g.tile([128, T], F32, tag="mb", name="mb")
        rsb = g.tile([128, T], F32, tag="rsb", name="rsb")

        # ---------------- Phase 1: LN1 -> XLn1 (bf16), pipelined ----------
        # x arrives in per-chunk DMAs; stats + apply are emitted per 392-col
        # chunk so the first qk matmuls can start ~15us in.
        # xln1 is stored WINDOW-ORDERED: col = w*196 + r*14 + c
        xln1 = []
        for ci in range(NC_C):
            xln1.append(pa.tile([128, T], BF16, tag=f"xln1_{ci}",
                                name=f"xln1_{ci}"))

        def emit_qk(ot, tts=None, epi_dve=True):
            hp2 = ot % 8
            tA, tB = ((qrelA, qrelB) if ot < 8 else (kindA, kindB))
            for tt in (range(NTT) if tts is None else tts):
                s = slice(tt * TT, (tt + 1) * TT)
                pst = pqkps.tile([128, TT], F32, tag="bigps", name="qkps",
                                 bufs=2)
                for ci in range(NC_C):
                    nc.tensor.matmul(
                        pst, wq[ci][:, ot * 128:(ot + 1) * 128],
                        xln1[ci][:, s],
                        start=(ci == 0), stop=(ci == NC_C - 1))
                nc.scalar.activation(out=tA[hp2 % 2][0:64, s],
                                     in_=pst[0:64, :],
                                     func=AF.Identity,
                                     bias=qkvb_sb[0:64, ot:ot + 1])
                if zero_bias and epi_dve:
                    nc.vector.tensor_copy(out=tB[hp2 % 2][64:128, s],
                                          in_=pst[64:128, :])
                else:
                    nc.scalar.activation(out=tB[hp2 % 2][64:128, s],
                                         in_=pst[64:128, :],
                                         func=AF.Identity,
                                         bias=qkvb_sb[64:128, ot:ot + 1])

        for tt in range(NTT):
            sl = slice(tt * TT, (tt + 1) * TT)
            sqs = []
            for ci in range(NC_C):
                sq = p1.tile([128, TT], BF16, tag=f"lnsq{ci}",
                             name=f"lnsq{ci}", bufs=2)
                nc.scalar.activation(out=sq, in_=xt[ci][:, sl],
                                     func=AF.Square)
                sqs.append(sq)
            ps_s = p1ps.tile([128, TT], F32, tag="lnsum", name="lnsum", bufs=2)
            ps_q = p1ps.tile([128, TT], F32, tag="lnsqp", name="lnsqp", bufs=2)
            for ci in range(NC_C):
                nc.tensor.matmul(ps_s, ones_bf, xt[ci][:, sl],
                                 start=(ci == 0), stop=(ci == NC_C - 1))
            for ci in range(NC_C):
                nc.tensor.matmul(ps_q, ones_bf, sqs[ci],
                                 start=(ci == 0), stop=(ci == NC_C - 1))
            nc.scalar.activation(out=mb[:, sl], in_=ps_s, func=AF.Copy,
                                 scale=1.0 / C)
            msq = p1.tile([128, TT], F32, tag="lnmsq", name="lnmsq", bufs=2)
            nc.vector.tensor_mul(out=msq, in0=mb[:, sl], in1=mb[:, sl])
            sqm = p1.tile([128, TT], F32, tag="lnsqm", name="lnsqm", bufs=2)
            nc.scalar.activation(out=sqm, in_=ps_q, func=AF.Copy,
                                 scale=1.0 / C)
            ve = p1.tile([128, TT], F32, tag="lnve", name="lnve", bufs=2)
            nc.vector.tensor_sub(out=ve, in0=sqm, in1=msq)
            nc.scalar.activation(out=ve, in_=ve, func=AF.Ln, bias=eps5)
            nc.scalar.activation(out=rsb[:, sl], in_=ve, func=AF.Exp,
                                 scale=-0.5)
            # apply: row-major (r, ww, c) -> window-ordered (ww, r, c)
            for ci in range(NC_C):
                tmp = p1.tile([128, TT], F32, tag="lntmp", name="lntmp",
                              bufs=3)
                nc.vector.tensor_sub(out=tmp, in0=xt[ci][:, sl],
                                     in1=mb[:, sl])
                nc.vector.tensor_mul(
                    out=xln1[ci][:, sl].rearrange(
                        "p (w r c) -> p w r c", w=2, r=WS),
                    in0=tmp.rearrange("p (r w c) -> p w r c", w=2, r=WS),
                    in1=rsb[:, sl].rearrange("p (r w c) -> p w r c",
                                             w=2, r=WS))
            # fuse the first head-pair's qk chains chunk-by-chunk so their
            # epilogues interleave with the LN1 applies on DVE/ACT
            emit_qk(0, tts=[tt])
            emit_qk(8, tts=[tt])
        dump("d_xln1", xln1)
        dump("d_mb", [mb])
        dump("d_rsb", [rsb])
        close(p1ps, p1, p1x)

        # ------- Block 1: qkv + attention, software-pipelined -------
        # Phase hp emits: qk matmuls for hp+1, then the window loop for hp
        # with rel-pos writeback for hp+1 (and v chains for s=1 on phases
        # 1-3) interleaved so the PE stream stays dense.
        pqk = pool("pqk", side="right")       # rotating qk tiles
        pvv = pool("pvv", side="right")       # v slices
        p2 = pool("p2", bufs=2)               # rel/pts/den tmps
        p2ps = pspool("p2ps", bufs=1)
        bvrow = pwq.tile([1, C], BF16, tag="bvrow", name="bvrow")
        nc.sync.dma_start(out=bvrow, in_=qkvbv[:, :])
        ones_row = pwq.tile([1, 98], BF16, tag="ones_row", name="ones_row")
        nc.vector.memset(ones_row, 1.0)

        xattn = []
        for hp in range(8):
            xattn.append(pc.tile([128, T], BF16, tag=f"xattn{hp}",
                                 name=f"xattn{hp}"))

        # kqind merge: stationary kind = [k rows | ind rows], moving qrel =
        # [q rows | rel rows]; one K=110/128 matmul replaces kq+ind pairs.
        #   qrelA/kindA rows: 0:64 q/k, 64:78 kh, 78:96 zero, 96:110 kw
        #   qrelB/kindB rows: 0:14 kh, 32:46 kw, 46:64 zero, 64:128 q/k
        def v_alloc(s):
            return pvv.tile([98, 16 * 512], BF16, tag=f"v{s}", name=f"v{s}")

        def emit_v_chain(s, vt, w, u):
            pv = pqkps.tile([98, 512], F32, tag="bigps", name="vps",
                            bufs=2)
            for ci in range(NC_C):
                nc.tensor.matmul(
                    pv, xln1[ci][:, w * 196 + u * 98:
                                 w * 196 + (u + 1) * 98],
                    wv[ci][:, s * 512:(s + 1) * 512],
                    start=(ci == 0),
                    stop=(zero_bias and ci == NC_C - 1))
            if not zero_bias:
                nc.tensor.matmul(
                    pv, ones_row[0:1, :],
                    bvrow[0:1, s * 512:(s + 1) * 512],
                    start=False, stop=True)
            dst = vt[:, (w * 2 + u) * 512:(w * 2 + u + 1) * 512]
            if (w * 2 + u) % 2 == 0:
                nc.scalar.copy(out=dst, in_=pv)
            else:
                nc.vector.tensor_copy(out=dst, in_=pv)

        def emit_rel_group(grp, qrA, qrB):
            # 4 rel-pos idx per PSUM bank; one strided writeback per
            # quadrant instead of one per idx.
            i0 = 4 * grp
            ni = min(4, WS - i0)
            qvA = qrA.rearrange("p (w a b) -> p w a b", w=N_WIN, a=WS)
            qvB = qrB.rearrange("p (w a b) -> p w a b", w=N_WIN, a=WS)
            rp = p2ps.tile([128, 448], F32, tag="relps", name="relps",
                           bufs=2)
            for il in range(ni):
                idx = i0 + il
                cs = slice(il * 112, (il + 1) * 112)
                nc.tensor.matmul(
                    rp[64:78, cs], rhe_sb[0:64, idx * 14:(idx + 1) * 14],
                    qvA[0:64, :, idx, :], start=True, stop=True,
                    tile_position=(0, 64))
                nc.tensor.matmul(
                    rp[96:110, cs], rwe_sb[0:64, idx * 14:(idx + 1) * 14],
                    qvA[0:64, :, :, idx], start=True, stop=True,
                    tile_position=(0, 96))
                nc.tensor.matmul(
                    rp[0:14, cs], rhe_sb[64:128, idx * 14:(idx + 1) * 14],
                    qvB[64:128, :, idx, :], start=True, stop=True,
                    tile_position=(64, 0))
                nc.tensor.matmul(
                    rp[32:46, cs], rwe_sb[64:128, idx * 14:(idx + 1) * 14],
                    qvB[64:128, :, :, idx], start=True, stop=True,
                    tile_position=(64, 32))
            rp5 = rp.rearrange("p (i w b) -> p i w b", i=4, w=N_WIN)[:, 0:ni]
            nc.vector.tensor_copy(
                out=qvA[64:78, :, i0:i0 + ni, :],
                in_=rp5[64:78].transpose([0, 2, 1, 3]))
            nc.vector.tensor_copy(
                out=qvA[96:110, :, :, i0:i0 + ni],
                in_=rp5[96:110].transpose([0, 2, 3, 1]))
            nc.scalar.copy(
                out=qvB[0:14, :, i0:i0 + ni, :],
                in_=rp5[0:14].transpose([0, 2, 1, 3]))
            nc.scalar.copy(
                out=qvB[32:46, :, :, i0:i0 + ni],
                in_=rp5[32:46].transpose([0, 2, 3, 1]))

        def emit_scores(hp, w, qrA, qrB, kA, kB):
            # one [98,392] score tile per head (both k-halves share a PSUM
            # bank) -> single exp per head
            base = w * 196
            pts = []
            for head in range(2):
                st = p2ps.tile([98, 392], F32, tag="stps", name="stps",
                               bufs=3)
                for u in range(2):
                    if head == 0:
                        nc.tensor.matmul(
                            st[:, u * 196:(u + 1) * 196],
                            kA[0:110, base + u * 98:base + (u + 1) * 98],
                            qrA[0:110, base:base + 196],
                            start=True, stop=True, tile_position=(0, 0))
                    else:
                        nc.tensor.matmul(
                            st[:, u * 196:(u + 1) * 196],
                            kB[0:128, base + u * 98:base + (u + 1) * 98],
                            qrB[0:128, base:base + 196],
                            start=True, stop=True, tile_position=(0, 0))
                pt = p2.tile([98, 392], BF16, tag="pt", name="pt", bufs=6)
                nc.scalar.activation(out=pt, in_=st, func=AF.Exp)
                pts.append(pt)
            return pts

        def emit_pv(hp, w, pts, vt, smb, pob):
            ptA, ptB = pts
            pvt = p2ps.tile([128, 196], F32, tag="pvps", name="pvps", bufs=1)
            smt = p2ps.tile([128, TT], F32, tag="stps", name="smps", bufs=3)
            for u in range(2):
                vbase = (w * 2 + u) * 512 + (hp % 4) * 128
                nc.tensor.matmul(
                    pvt[0:64, :], vt[:, vbase:vbase + 64],
                    ptA[:, u * 196:(u + 1) * 196],
                    start=(u == 0), stop=(u == 1),
                    tile_position=(0, 0), skip_group_check=True)
                nc.tensor.matmul(
                    pvt[64:128, :], vt[:, vbase + 64:vbase + 128],
                    ptB[:, u * 196:(u + 1) * 196],
                    start=(u == 0), stop=(u == 1),
                    tile_position=(0, 64), skip_group_check=True)
            for u in range(2):
                nc.tensor.matmul(
                    smt[:, 0:196], ones_bf[0:98, :],
                    ptA[:, u * 196:(u + 1) * 196],
                    start=(u == 0), stop=(u == 1), skip_group_check=True)
            for u in range(2):
                nc.tensor.matmul(
                    smt[:, 196:392], ones_bf[0:98, :],
                    ptB[:, u * 196:(u + 1) * 196],
                    start=(u == 0), stop=(u == 1), skip_group_check=True)
            nc.vector.tensor_copy(out=smb[:, w * TT:(w + 1) * TT], in_=smt)
            nc.vector.tensor_copy(out=pob[:, w * 196:(w + 1) * 196], in_=pvt)

        def emit_den_muls(hp, smb, pob):
            # batched softmax denominators for all 8 windows of this hp:
            # 1/s = exp(-ln(s)); rs2b packs head A on parts 0:64, B on 64:128
            nc.scalar.activation(out=smb, in_=smb, func=AF.Ln)
            lt4 = smb.rearrange("p (w h q) -> p w h q", w=N_WIN, h=2)
            rs2b = p2.tile([128, 8 * 196], F32, tag="rs2b", name="rs2b",
                           bufs=1)
            rs4 = rs2b.rearrange("p (w q) -> p w q", w=N_WIN)
            nc.scalar.activation(out=rs4[0:64, :, :], in_=lt4[0:64, :, 0, :],
                                 func=AF.Exp, scale=-1.0)
            nc.scalar.activation(out=rs4[64:128, :, :],
                                 in_=lt4[64:128, :, 1, :],
                                 func=AF.Exp, scale=-1.0)
            for w in range(N_WIN):
                nc.vector.tensor_mul(
                    out=win_view(xattn[hp], w),
                    in0=pob[:, w * 196:(w + 1) * 196].rearrange(
                        "p (r c) -> p r c", r=WS),
                    in1=rs2b[:, w * 196:(w + 1) * 196].rearrange(
                        "p (r c) -> p r c", r=WS))

        def emit_windows(hp, rel_hp, vchains):
            qrA, qrB = qrelA[hp % 2], qrelB[hp % 2]
            kA, kB = kindA[hp % 2], kindB[hp % 2]
            vt = v_tiles[hp // 4]
            smb = p2.tile([128, 8 * TT], F32, tag="smb", name="smb", bufs=1)
            pob = p2.tile([128, 8 * 196], F32, tag="pob", name="pob", bufs=2)
            rel_steps = {1: 0, 3: 1, 5: 2, 7: 3} if rel_hp is not None else {}
            vchains = list(vchains)
            pts_q = {}
            for w in range(N_WIN + 2):
                if w < N_WIN:
                    pts_q[w] = emit_scores(hp, w, qrA, qrB, kA, kB)
                if w >= 2:
                    emit_pv(hp, w - 2, pts_q.pop(w - 2), vt, smb, pob)
                if w in rel_steps:
                    emit_rel_group(rel_steps[w],
                                   qrelA[rel_hp % 2], qrelB[rel_hp % 2])
                for _ in range(min(2, len(vchains))):
                    vs, vw, vu = vchains.pop(0)
                    emit_v_chain(vs, v_tiles[vs], vw, vu)
            emit_den_muls(hp, smb, pob)

        v_tiles = {}
        v_tiles[0] = v_alloc(0)
        for grp in range(4):
            emit_rel_group(grp, qrelA[0], qrelB[0])
        v_sched = {0: [(0, w, u) for w in range(N_WIN) for u in (0, 1)],
                   1: [(1, w, u) for w in (0, 1, 2) for u in (0, 1)],
                   2: [(1, w, u) for w in (3, 4, 5) for u in (0, 1)],
                   3: [(1, w, u) for w in (6, 7) for u in (0, 1)]}
        for hp in range(8):
            if hp + 1 < 8:
                emit_qk(hp + 1)
                emit_qk(9 + hp)
            if hp == 1:
                v_tiles[1] = v_alloc(1)
            emit_windows(hp, rel_hp=hp + 1 if hp + 1 < 8 else None,
                         vchains=v_sched.get(hp, []))
        dump("d_xattn", xattn)
        close(p2ps, pqkps, p2, pvv, pqk, pwq)

        # ---------------- Phase 3: proj + residual ----------------
        px2 = pool("px2", side="right")                     # x2, lives to the end
        p3 = pool("p3", bufs=2)
        p3ps = pspool("p3ps", bufs=2)
        x2 = []
        for ot in range(NC_C):
            x2.append(px2.tile([128, T], BF16,
                               tag=f"x2_{ot}", name=f"x2_{ot}"))
        wp = []
        for ci in range(NC_C):
            t = p3.tile([128, C], BF16, tag=f"wproj{ci}", name=f"wproj{ci}",
                        bufs=1)
            nc.sync.dma_start(out=t, in_=projwT[ci * 128:(ci + 1) * 128, :])
            wp.append(t)
        for tt in range(NTT):
            for ot in range(NC_C):
                pst = p3ps.tile([128, TT], F32, tag="projps", name="projps", bufs=3)
                for ci in range(NC_C):
                    nc.tensor.matmul(
                        pst, wp[ci][:, ot * 128:(ot + 1) * 128],
                        xattn[ci][:, tt * TT:(tt + 1) * TT],
                        start=(ci == 0), stop=(ci == NC_C - 1))
                tmp = p3.tile([128, TT], F32, tag="projtmp", name="projtmp")
                nc.scalar.activation(out=tmp, in_=pst, func=AF.Identity,
                                     bias=projb_sb[:, ot:ot + 1])
                xre = p3.tile([128, TT], BF16, tag="xre", name="xre")
                nc.sync.dma_start(
                    out=xre,
                    in_=xT[ot * 128:(ot + 1) * 128, tt * TT:(tt + 1) * TT])
                nc.vector.tensor_add(
                    out=x2[ot][:, tt * TT:(tt + 1) * TT], in0=tmp, in1=xre)
        dump("d_x2", x2)
        close(p3ps, p3, pc, pa)

        # ---------------- Phase 4: LN2 + MLP ----------------
        p4x = pool("p4x")                     # xln2
        p4w = pool("p4w", bufs=2)             # fc weight rings (DMA early)
        p4a = pool("p4a", bufs=2)
        p4aps = pspool("p4aps", bufs=2)

        def load_w1(og):
            w1 = []
            for ci in range(NC_C):
                t = p4w.tile([128, 512], BF16, tag=f"w1_{ci}",
                             name=f"w1_{ci}", bufs=2)
                nc.sync.dma_start(
                    out=t, in_=fc1wT[ci * 128:(ci + 1) * 128,
                                     og * 512:(og + 1) * 512])
                w1.append(t)
            return w1

        w1_pre = {0: load_w1(0), 1: load_w1(1)}
        xln2 = []
        for ci in range(NC_C):
            xln2.append(p4x.tile([128, T], BF16, tag=f"xln2_{ci}",
                                 name=f"xln2_{ci}"))

        def apply_ln2(sl, pl):
            for ci in range(NC_C):
                tmp = pl.tile([128, 392], F32, tag="lntmp", name="lntmp",
                              bufs=3)
                nc.vector.tensor_sub(out=tmp, in0=x2[ci][:, sl],
                                     in1=mb[:, sl])
                nc.vector.tensor_mul(out=xln2[ci][:, sl], in0=tmp,
                                     in1=rsb[:, sl])

        # stats for all chunks, but apply only chunks 0/1 now: fc1's tp=0
        # pass needs just those, so chunks 2/3 apply under the fc1 og loop
        for tt in range(NTT):
            sl = slice(tt * TT, (tt + 1) * TT)
            ln_chunk(nc, p4a, p4aps, x2, sl, mb, rsb, 1.0 / C, eps5)
            if tt < 2:
                apply_ln2(sl, p4a)
        dump("d_xln2", xln2)
        close(p4aps, p4a)

        p4 = pool("p4", bufs=2)
        p4ps = pspool("p4ps", bufs=2)
        hbuf = p4.tile([128, 32 * 784], BF16, tag="hbuf", name="hbuf", bufs=1)
        for tp in range(2):
            for og in range(8):
                w1 = (w1_pre[og] if (tp == 0 and og in w1_pre)
                      else load_w1(og))
                for tl in range(2):
                    for otl in range(4):
                        ot = og * 4 + otl
                        t0 = tp * 784 + tl * TT
                        pst = p4ps.tile([128, TT], F32, tag="fc1ps",
                                        name="fc1ps", bufs=3)
                        for ci in range(NC_C):
                            nc.tensor.matmul(
                                pst, w1[ci][:, otl * 128:(otl + 1) * 128],
                                xln2[ci][:, t0:t0 + TT],
                                start=(ci == 0), stop=(ci == NC_C - 1))
                        nc.scalar.activation(
                            out=hbuf[:, ot * 784 + tl * TT:
                                     ot * 784 + (tl + 1) * TT],
                            in_=pst, func=AF.Gelu,
                            bias=fc1b_sb[:, ot:ot + 1])
                if tp == 0 and og < 2:
                    apply_ln2(slice((2 + og) * TT, (3 + og) * TT), p4w)
            for og2 in range(4):
                w2 = []
                for ki in range(32):
                    t = p4w.tile([128, 256], BF16, tag=f"w2_{ki}",
                                 name=f"w2_{ki}", bufs=2)
                    nc.sync.dma_start(
                        out=t, in_=fc2wT[ki * 128:(ki + 1) * 128,
                                         og2 * 256:(og2 + 1) * 256])
                    w2.append(t)
                for otl in range(2):
                    ot = og2 * 2 + otl
                    for tl in range(2):
                        t0 = tp * 784 + tl * TT
                        pst = p4ps.tile([128, TT], F32, tag="fc2ps",
                                        name="fc2ps", bufs=3)
                        for ki in range(32):
                            nc.tensor.matmul(
                                pst, w2[ki][:, otl * 128:(otl + 1) * 128],
                                hbuf[:, ki * 784 + tl * TT:
                                     ki * 784 + (tl + 1) * TT],
                                start=(ki == 0), stop=(ki == 31))
                        tmp = p4.tile([128, TT], F32, tag="fc2tmp",
                                      name="fc2tmp")
                        nc.scalar.activation(
                            out=tmp, in_=pst, func=AF.Identity,
                            bias=fc2b_sb[:, ot:ot + 1])
                        nc.vector.tensor_add(
                            out=x2[ot][:, t0:t0 + TT],
                            in0=tmp, in1=x2[ot][:, t0:t0 + TT])
        if debug:
            for ki in range(32):
                nc.sync.dma_start(out=dbg["d_h"][ki * 128:(ki + 1) * 128, :],
                                  in_=hbuf[:, ki * 784:(ki + 1) * 784])
        dump("d_x3", x2)
        close(p4ps, p4, p4w, p4x)

        # ---------------- Phase 5: ResBottleneckBlock ----------------
        pcw = pool("pcw")                     # all conv weights, DMA'd early
        wc1 = []
        for ci in range(NC_C):
            t = pcw.tile([128, 512], BF16, tag=f"wc1_{ci}", name=f"wc1_{ci}")
            nc.sync.dma_start(out=t, in_=c1wT[ci * 128:(ci + 1) * 128, :])
            wc1.append(t)
        wc2 = {}
        for tap in range(9):
            for ci in range(4):
                t = pcw.tile([128, 512], BF16, tag=f"wc2_{tap}_{ci}",
                             name=f"wc2_{tap}_{ci}")
                nc.sync.dma_start(
                    out=t, in_=c2wT[tap, ci * 128:(ci + 1) * 128, :])
                wc2[(tap, ci)] = t
        wc3 = []
        for ci in range(4):
            t = pcw.tile([128, C], BF16, tag=f"wc3_{ci}", name=f"wc3_{ci}")
            nc.sync.dma_start(out=t, in_=c3wT[ci * 128:(ci + 1) * 128, :])
            wc3.append(t)
        pr1 = pool("pr1")                     # r1pad, lives 5a..5b
        r1pad = []
        for ci in range(4):
            t = pr1.tile([128, BS * 900 + 2], BF16, tag=f"r1pad{ci}",
                         name=f"r1pad{ci}")
            nc.gpsimd.memset(t, 0.0)
            r1pad.append(t)
        p5a = pool("p5a", bufs=2)
        p5aps = pspool("p5aps", bufs=2)
        c1s = [p5a.tile([128, T], BF16, tag=f"c1s{ot}",
                        name=f"c1s{ot}", bufs=1) for ot in range(4)]
        def apply_n1(sl):
            tt = sl.start // TT
            i, wh = tt // 2, tt % 2
            for ci in range(4):
                pv = r1pad[ci][:, 0:1800].rearrange(
                    "p (i y x) -> p i y x", i=BS, y=30)
                tmp = p5a.tile([128, TT], F32, tag="c1tmp", name="c1tmp",
                               bufs=3)
                nc.vector.tensor_sub(out=tmp, in0=c1s[ci][:, sl],
                                     in1=mb[:, sl])
                nc.vector.tensor_mul(out=tmp, in0=tmp, in1=rsb[:, sl])
                nc.scalar.activation(
                    out=pv[:, i, 1 + 14 * wh:15 + 14 * wh, 1:29],
                    in_=tmp, func=AF.Gelu,
                    bias=n1b_sb[:, ci:ci + 1], scale=n1w_sb[:, ci:ci + 1])

        for tt in range(NTT):
            sl = slice(tt * TT, (tt + 1) * TT)
            for ot in range(4):
                pst = p5aps.tile([128, TT], F32, tag="c1ps", name="c1ps",
                                 bufs=3)
                for ci in range(NC_C):
                    nc.tensor.matmul(
                        pst, wc1[ci][:, ot * 128:(ot + 1) * 128],
                        x2[ci][:, sl],
                        start=(ci == 0), stop=(ci == NC_C - 1))
                nc.vector.tensor_copy(out=c1s[ot][:, sl], in_=pst)
            ln_chunk(nc, p5a, p5aps, c1s, sl, mb, rsb, 1.0 / 512, eps6)
            apply_n1(sl)
        dump("d_c1", c1s)
        dump("d_r1p", r1pad)
        close(p5aps, p5a)

        pr2 = pool("pr2", side="right")                     # r2, lives 5b..5c
        p5b = pool("p5b", bufs=2)
        p5bps = pspool("p5bps", bufs=2)
        c2s = [p5b.tile([128, T], BF16, tag=f"c2s{ot}",
                        name=f"c2s{ot}", bufs=1) for ot in range(4)]
        def conv2_chunk(tt):
            for ot in range(4):
                i, wh = tt // 2, tt % 2
                pst = p5bps.tile([128, 420], F32, tag="c2ps", name="c2ps", bufs=3)
                n9 = 0
                for tap in range(9):
                    dy, dx = tap // 3, tap % 3
                    for ci in range(4):
                        st0 = i * 900 + (14 * wh + dy) * 30 + dx
                        nc.tensor.matmul(
                            pst, wc2[(tap, ci)][:, ot * 128:(ot + 1) * 128],
                            r1pad[ci][:, st0:st0 + 420],
                            start=(n9 == 0), stop=(n9 == 35))
                        n9 += 1
                nc.scalar.copy(
                    out=c2s[ot][:, tt * TT:(tt + 1) * TT].rearrange(
                        "p (h x) -> p h x", h=WS),
                    in_=pst.rearrange("p (h x) -> p h x", h=WS)[:, :, 0:28])
        r2 = []
        for ci in range(4):
            r2.append(pr2.tile([128, T], BF16, tag=f"r2_{ci}",
                               name=f"r2_{ci}"))

        def apply_n2(sl):
            for ci in range(4):
                tmp = p5b.tile([128, TT], F32, tag="c2tmp", name="c2tmp",
                               bufs=3)
                nc.vector.tensor_sub(out=tmp, in0=c2s[ci][:, sl],
                                     in1=mb[:, sl])
                nc.vector.tensor_mul(out=tmp, in0=tmp, in1=rsb[:, sl])
                nc.scalar.activation(
                    out=r2[ci][:, sl], in_=tmp, func=AF.Gelu,
                    bias=n2b_sb[:, ci:ci + 1], scale=n2w_sb[:, ci:ci + 1])

        for tt in range(NTT):
            sl = slice(tt * TT, (tt + 1) * TT)
            conv2_chunk(tt)
            ln_chunk(nc, p5b, p5bps, c2s, sl, mb, rsb, 1.0 / 512, eps6)
            apply_n2(sl)
        dump("d_c2", c2s)
        close(p5bps, p5b, pr1)

        p5c = pool("p5c", bufs=2)
        p5cps = pspool("p5cps", bufs=2)
        c3s = [p5c.tile([128, T], BF16, tag=f"c3s{ot}",
                        name=f"c3s{ot}", bufs=1) for ot in range(NC_C)]
        def apply_n3(s):
            # spread across DVE (sub/mul/scale), gpsimd (residual)
            for ot in range(NC_C):
                tmp = p5c.tile([128, s.stop - s.start], F32, tag="ytmp",
                               name="ytmp", bufs=3)
                nc.vector.tensor_sub(out=tmp, in0=c3s[ot][:, s], in1=mb[:, s])
                nc.vector.tensor_mul(out=tmp, in0=tmp, in1=rsb[:, s])
                if zero_bias:
                    nc.vector.tensor_scalar_mul(out=tmp, in0=tmp,
                                                scalar1=n3w_sb[:, ot:ot + 1])
                else:
                    nc.scalar.activation(
                        out=tmp, in_=tmp, func=AF.Identity,
                        scale=n3w_sb[:, ot:ot + 1], bias=n3b_sb[:, ot:ot + 1])
                yt = p5c.tile([128, s.stop - s.start], F32, tag="yt",
                              name="yt", bufs=3)
                nc.gpsimd.tensor_add(out=yt, in0=tmp, in1=x2[ot][:, s])
                nc.sync.dma_start(out=yT[ot * 128:(ot + 1) * 128, s], in_=yt)

        for tt in range(NTT):
            sl = slice(tt * TT, (tt + 1) * TT)
            for ot in range(NC_C):
                pst = p5cps.tile([128, TT], F32, tag="c3ps", name="c3ps",
                                 bufs=3)
                for ci in range(4):
                    nc.tensor.matmul(
                        pst, wc3[ci][:, ot * 128:(ot + 1) * 128],
                        r2[ci][:, sl],
                        start=(ci == 0), stop=(ci == 3))
                nc.scalar.copy(out=c3s[ot][:, sl], in_=pst)
            if tt < NTT - 1:
                ln_chunk(nc, p5c, p5cps, c3s, sl, mb, rsb, 1.0 / C, eps6)
                apply_n3(sl)
            else:
                # final chunk: two 196-col halves shorten the closing
                # stats->apply->store chain
                for h in range(2):
                    hs = slice(sl.start + h * 196, sl.start + (h + 1) * 196)
                    ln_chunk(nc, p5c, p5cps, c3s, hs, mb, rsb, 1.0 / C, eps6)
                    apply_n3(hs)
        dump("d_c3", c3s)
        close(p5cps, p5c, pcw, pr2, px2, g)

    return nc


# ---------------------------------------------------------------------------
# Host side
# ---------------------------------------------------------------------------
_program_cache = {}


def _get_program(zero_bias=False):
    key = ("nc", zero_bias)
    if key not in _program_cache:
        _program_cache[key] = build_program(zero_bias=zero_bias)
    return _program_cache[key]


def _bf(x):
    return np.ascontiguousarray(x).astype(ml_dtypes.bfloat16)


def prep_inputs(inputs):
    """Build the per-core input maps (host-side sharding + weight prep)."""
    f = {k: np.asarray(v, dtype=np.float32) for k, v in inputs.items()}
    scale = HD ** -0.5

    qkv_w = f["qkv_w"].copy()          # [3C, C]
    qkv_b = f["qkv_b"].copy()          # [3C]
    qkv_w[:C] *= scale                 # fold 1/sqrt(hd) into q
    qkv_b[:C] *= scale
    # fold ln1 affine into qkv
    qkv_wT = (qkv_w * f["ln1_w"][None, :]).T.copy()      # [C, 3C]
    qkv_b_eff = qkv_b + qkv_w @ f["ln1_b"]
    # fold ln2 affine into fc1
    fc1_wT = (f["fc1_w"] * f["ln2_w"][None, :]).T.copy()  # [C, MLP]
    fc1_b_eff = f["fc1_b"] + f["fc1_w"] @ f["ln2_b"]

    proj_wT = f["proj_w"].T.copy()
    fc2_wT = f["fc2_w"].T.copy()
    c1_wT = f["conv1_w"][:, :, 0, 0].T.copy()            # [C, C/2]
    c2 = f["conv2_w"]                                    # [O, I, 3, 3]
    c2_wT = np.ascontiguousarray(
        c2.transpose(2, 3, 1, 0).reshape(9, C // 2, C // 2))
    c3_wT = f["conv3_w"][:, :, 0, 0].T.copy()            # [C/2, C]

    # rel pos tables: rhe[p, qh*14+kh] = 8*rel_pos_h[qh-kh+13, p%64]
    rh8 = 8.0 * f["rel_pos_h"]                           # [27, 64]
    rw8 = 8.0 * f["rel_pos_w"]
    qh_i, kh_i = np.meshgrid(np.arange(WS), np.arange(WS), indexing="ij")
    idx = qh_i - kh_i + WS - 1                           # [qh, kh]
    rhe = rh8[idx]                                       # [qh, kh, 64]
    rwe = rw8[idx]
    rhe_t = np.zeros((128, 196), np.float32)
    rwe_t = np.zeros((128, 196), np.float32)
    rhe_flat = rhe.transpose(2, 0, 1).reshape(64, 196)   # [c, qh*14+kh]
    rwe_flat = rwe.transpose(2, 0, 1).reshape(64, 196)
    rhe_t[0:64] = rhe_flat
    rhe_t[64:128] = rhe_flat
    rwe_t[0:64] = rwe_flat
    rwe_t[64:128] = rwe_flat

    # indicators vs rel rows: A-kh 0:14, B-kh 32:46, A-kw 64:78, B-kw 96:110
    indA = np.zeros((128, 196), np.float32)
    indB = np.zeros((128, 196), np.float32)
    kt = np.arange(196)
    for j in range(WS):
        indA[j, kt // 14 == j] = 1.0
        indA[64 + j, kt % 14 == j] = 1.0
        indB[32 + j, kt // 14 == j] = 1.0
        indB[96 + j, kt % 14 == j] = 1.0
    # device-side kind/qrel init templates (rows 64:110 of kindA, 0:64 of
    # kindB, plus an 18-row zero slab for the qrel pad rows)
    kindA_init = np.zeros((46, T), np.float32)
    kindB_init = np.zeros((64, T), np.float32)
    for w in range(8):
        cs = slice(w * 196, (w + 1) * 196)
        kindA_init[0:14, cs] = indA[0:14]
        kindA_init[32:46, cs] = indA[64:78]
        kindB_init[0:14, cs] = indB[32:46]
        kindB_init[32:46, cs] = indB[96:110]

    common = {
        "qkvwT": _bf(qkv_wT),
        "qkvb": qkv_b_eff.astype(np.float32),
        "qkvbv": _bf(qkv_b_eff[2 * C:][None, :]),
        "projwT": _bf(proj_wT),
        "projb": f["proj_b"],
        "fc1wT": _bf(fc1_wT),
        "fc1b": fc1_b_eff.astype(np.float32),
        "fc2wT": _bf(fc2_wT),
        "fc2b": f["fc2_b"],
        "c1wT": _bf(c1_wT),
        "c2wT": _bf(c2_wT),
        "c3wT": _bf(c3_wT),
        "n1w": f["n1_w"], "n1b": f["n1_b"],
        "n2w": f["n2_w"], "n2b": f["n2_b"],
        "n3w": f["n3_w"], "n3b": f["n3_b"],
        "rhe": _bf(rhe_t), "rwe": _bf(rwe_t),
        "kindAi": _bf(kindA_init), "kindBi": _bf(kindB_init),
        "zpad": np.zeros((18, T), ml_dtypes.bfloat16),
    }
    x = f["x"]                                           # [B, 28, 28, C]
    in_maps = []
    for core in range(NCORES):
        xs = x[core * BS:(core + 1) * BS].reshape(T, C).T  # [C, T]
        m = dict(common)
        m["xT"] = _bf(xs)
        in_maps.append(m)
    return in_maps


def run(inputs, trace=False):
    from concourse.bass_utils import run_bass_kernel_spmd

    in_maps = prep_inputs(inputs)
    zb = bool(np.all(in_maps[0]["qkvb"] == 0.0)
              and np.all(in_maps[0]["n3b"] == 0.0))
    nc = _get_program(zero_bias=zb)
    res = run_bass_kernel_spmd(nc, in_maps, core_ids=list(range(NCORES)),
                               trace=trace)
    outs = []
    for core in range(NCORES):
        yt = res.results[core]["yT"]                     # [C, T]
        outs.append(yt.T.reshape(BS, HH, WW, C))
    y = np.concatenate(outs, axis=0).astype(np.float32)
    return y, res


def kernel(**inputs):
    y, _ = run(inputs, trace=False)
    return y

